# revision 22
# baseline (speedup 1.0000x reference)
"""Trainium2 Bass kernel for causal multi-head attention with NeoX RoPE.

Problem: x[2, 2048, 1024], 16 heads x d_head 64, rotary over all 64 dims,
causal softmax, output projection.

Sharding (v2): every core holds 2 heads ({2c, 2c+1}) and processes BOTH
batches.  The output projection is computed after a per-q-chunk 8-core
AllToAll of the normalized z shards: core c sends z[b, qsub, its 2 heads]
to the core that owns output rows (b, qsub); each core then contracts all
16 heads locally (W_O resident, slot s = heads {2s, 2s+1}) and writes its
own [128 x 1024] output rows per chunk.  This replaces the f16-partial
ReduceScatter chain of v1 (4x less collective traffic, no serial RS tail).

Per-core dataflow (batch b, 2 heads packed on 128 partitions):
  xT [b, d, s] (bf16, host-transposed)
  Q/K projections -> [128, s] via PE; RoPE as q*cos + flip(q)*sin' with the
    rotate-half flip fused into per-32-partition-block multiplies and the
    flip sign folded into the sin table.
  V projection -> Vs [s, h, 65] with a ones column (softmax denominator
    rides the AV matmul for free).
  Scores transposed: S_T[k, q] = kT.T @ qT per 128-k-tile; the two heads run
    as concurrent 64x128 PE row-tiles; exp on ScalarE (scale 1/8 folded in);
    causal mask via narrow GPSIMD affine_select on the 128x128 diagonal
    blocks only.
  AV: z[65, q] += V_aug.T @ E per k-tile; row 64 is the denominator.
  Normalize via reciprocal_approx_fast + partition_broadcast + multiply.
  AllToAll(z) per chunk, then out[q, m] = zall.T @ W_O locally.
"""

import numpy as np
import ml_dtypes

S = 2048
D = 1024
NH = 16
DH = 64
SCALE = 8.0
ROT_BASE = 10000.0
N_CORES = 8
QCHUNK = 512     # q chunk (free dim of score matmuls)
NCHUNK = S // QCHUNK
KTILE = 128
BF = ml_dtypes.bfloat16
GROUPS_ALL = [[0, 1, 2, 3, 4, 5, 6, 7]]

_BUILT = {}


def _build(with_qk_bias):
    import concourse.bass as bass
    import concourse.tile as tile
    from concourse import bacc, mybir

    f32 = mybir.dt.float32
    bf16 = mybir.dt.bfloat16
    f16 = mybir.dt.float16
    AF = mybir.ActivationFunctionType
    OP = mybir.AluOpType

    nc = bacc.Bacc("TRN2", target_bir_lowering=False, debug=False,
                   num_devices=N_CORES)

    xT = nc.dram_tensor("xT", [2, 128, 8, S], bf16, kind="ExternalInput").ap()
    wq = nc.dram_tensor("wq", [128, 8, 128], bf16, kind="ExternalInput").ap()
    wk = nc.dram_tensor("wk", [128, 8, 128], bf16, kind="ExternalInput").ap()
    wv = nc.dram_tensor("wv", [128, 8, 128], bf16, kind="ExternalInput").ap()
    wo = nc.dram_tensor("wo", [128, 8, D], bf16, kind="ExternalInput").ap()
    cosd = nc.dram_tensor("cosT", [128, S], bf16, kind="ExternalInput").ap()
    sind = nc.dram_tensor("sinTm", [128, S], bf16, kind="ExternalInput").ap()
    if with_qk_bias:
        bqd = nc.dram_tensor("bq", [128, 1], f32, kind="ExternalInput").ap()
        bkd = nc.dram_tensor("bk", [128, 1], f32, kind="ExternalInput").ap()

    z_send = [nc.dram_tensor(f"z_send{j}", [8, 128, 128], bf16)
              for j in range(NCHUNK)]
    z_recv = [nc.dram_tensor(f"z_recv{j}", [8, 128, 128], bf16)
              for j in range(NCHUNK)]
    out_ext = nc.dram_tensor("out_shard", [S // 4, D], f16,
                             kind="ExternalOutput").ap()

    with tile.TileContext(nc) as tc:
        with (
            tc.tile_pool(name="consts", bufs=1) as consts,
            tc.tile_pool(name="qk", bufs=1) as qkpool,
            tc.tile_pool(name="vsb", bufs=1) as vpool,
            tc.tile_pool(name="rope", bufs=2) as rope,
            tc.tile_pool(name="epool", bufs=2) as epool,
            tc.tile_pool(name="zpool", bufs=4) as zpool,
            tc.tile_pool(name="den", bufs=2) as den,
            tc.tile_pool(name="zail", bufs=2) as zallp,
            tc.tile_pool(name="osb", bufs=2) as osb,
            tc.tile_pool(name="ps_sc", bufs=2, space="PSUM") as ps_sc,
            tc.tile_pool(name="ps_av", bufs=2, space="PSUM") as ps_av,
            tc.tile_pool(name="ps_pj", bufs=2, space="PSUM") as ps_pj,
        ):
            wq_sb = consts.tile([128, 8, 128], bf16, tag="wq")
            nc.gpsimd.dma_start(out=wq_sb, in_=wq)
            wk_sb = consts.tile([128, 8, 128], bf16, tag="wk")
            nc.gpsimd.dma_start(out=wk_sb, in_=wk)
            cos_sb = consts.tile([128, S], bf16, tag="cos")
            nc.gpsimd.dma_start(out=cos_sb, in_=cosd)
            sin_sb = consts.tile([128, S], bf16, tag="sin")
            nc.gpsimd.dma_start(out=sin_sb, in_=sind)
            wv_sb = consts.tile([128, 8, 128], bf16, tag="wv")
            nc.gpsimd.dma_start(out=wv_sb, in_=wv)
            wo_sb = consts.tile([128, 8, D], bf16, tag="wo")
            nc.gpsimd.dma_start(out=wo_sb, in_=wo)

            xT_sb = consts.tile([128, 2, 8, S], bf16, tag="xT")
            # first-needed slices first: batch 0 cols 0:512, then the rest
            for kt in range(8):
                nc.sync.dma_start(out=xT_sb[:, 0, kt, 0:512],
                                  in_=xT[0][:, kt, 0:512])
            for kt in range(8):
                nc.sync.dma_start(out=xT_sb[:, 0, kt, 512:S],
                                  in_=xT[0][:, kt, 512:S])
            for kt in range(8):
                nc.sync.dma_start(out=xT_sb[:, 1, kt, :], in_=xT[1][:, kt, :])
            if with_qk_bias:
                bq_sb = consts.tile([128, 1], f32, tag="bq")
                nc.sync.dma_start(out=bq_sb, in_=bqd)
                bk_sb = consts.tile([128, 1], f32, tag="bk")
                nc.sync.dma_start(out=bk_sb, in_=bkd)

            warm = consts.tile([128, 8], f32, tag="warm")
            nc.vector.memset(warm, 0.0)
            nc.scalar.activation(out=warm, in_=warm, func=AF.Exp, scale=1.0)

            # Persistent rotated Q/K: [128 (=2-head pack), batch, s]
            Qr = qkpool.tile([128, 2, S], bf16, tag="Qr")
            Kr = qkpool.tile([128, 2, S], bf16, tag="Kr")
            # V with ones column: [s-part, batch, s-tile, head, 65]
            Vs = vpool.tile([128, 2, 16, 2, 65], bf16, tag="Vs")
            nc.vector.memset(Vs[:, :, :, :, 64:65], 1.0)

            # ---- per-chunk projections (interleaved with attention) ----
            def proj_chunk(b, c):
                cs = slice(c * QCHUNK, (c + 1) * QCHUNK)
                for (wsb, bias_name, dst) in (
                        (wq_sb, "bq", Qr), (wk_sb, "bk", Kr)):
                    pt = ps_pj.tile([128, QCHUNK], f32, tag="pj")
                    for kt in range(8):
                        nc.tensor.matmul(
                            out=pt, lhsT=wsb[:, kt, :],
                            rhs=xT_sb[:, b, kt, cs],
                            start=(kt == 0), stop=(kt == 7))
                    if with_qk_bias:
                        bsb = bq_sb if bias_name == "bq" else bk_sb
                        nc.vector.tensor_scalar_add(
                            out=pt, in0=pt, scalar1=bsb[:, 0:1])
                    q_sb = rope.tile([128, QCHUNK], bf16, tag="ropeA")
                    nc.vector.tensor_copy(out=q_sb, in_=pt)
                    # q_rot = q*cos + flip(q)*sin' (sign folded into sin')
                    qf = rope.tile([128, QCHUNK], bf16, tag="ropeB")
                    for blk in range(4):
                        src = (blk ^ 1) * 32
                        nc.vector.tensor_copy(
                            out=qf[blk * 32:blk * 32 + 32, :],
                            in_=q_sb[src:src + 32, :])
                    qs = rope.tile([128, QCHUNK], bf16, tag="ropeC")
                    nc.vector.tensor_tensor(
                        out=qs, in0=qf, in1=sin_sb[:, cs], op=OP.mult)
                    qc = rope.tile([128, QCHUNK], bf16, tag="ropeB")
                    nc.vector.tensor_tensor(
                        out=qc, in0=q_sb, in1=cos_sb[:, cs], op=OP.mult)
                    nc.vector.tensor_tensor(
                        out=dst[:, b, cs], in0=qc, in1=qs, op=OP.add)

            def proj_v(b, c):
                for st in range(4 * c, 4 * c + 4):
                    pt = ps_pj.tile([128, 2, 64], f32, tag="pj")
                    for kt in range(8):
                        nc.tensor.matmul(
                            out=pt,
                            lhsT=xT_sb[:, b, kt, st * 128:(st + 1) * 128],
                            rhs=wv_sb[:, kt, :],
                            start=(kt == 0), stop=(kt == 7))
                    nc.vector.tensor_copy(
                        out=Vs[:, b, st, :, 0:64], in_=pt)

            # ---- attention (softmax pipelined with PE) ----
            E_tiles = {}
            zsb_tiles = {}

            def scores_exp(b, j):
                nkt = 4 * j + 4
                E = epool.tile([128, 16, 2, QCHUNK], bf16, tag="E")
                E_tiles[(b, j)] = E
                for t in range(nkt):
                    q0 = max(0, 128 * (t - 4 * j))
                    qs2 = slice(j * QCHUNK + q0, (j + 1) * QCHUNK)
                    sc = ps_sc.tile([128, 2, QCHUNK], f32, tag="sc")
                    for hh in range(2):
                        hs = slice(64 * hh, 64 * hh + 64)
                        nc.tensor.matmul(
                            out=sc[:, hh, q0:],
                            lhsT=Kr[hs, b, t * 128:(t + 1) * 128],
                            rhs=Qr[hs, b, qs2], start=True, stop=True)
                    nc.scalar.activation(
                        out=E[:, t, :, q0:], in_=sc[:, :, q0:],
                        func=AF.Exp, scale=1.0 / SCALE)

            def mask(b, j):
                E = E_tiles[(b, j)]
                for dt in range(4):
                    t = 4 * j + dt
                    qb = slice(128 * dt, 128 * dt + 128)
                    nc.gpsimd.affine_select(
                        out=E[:, t, :, qb], in_=E[:, t, :, qb],
                        pattern=[[0, 2], [1, 128]], base=0,
                        channel_multiplier=-1,
                        compare_op=OP.is_ge, fill=0.0)

            def av_normalize(b, j):
                nkt = 4 * j + 4
                E = E_tiles.pop((b, j))
                zsb = zpool.tile([128, 4, 128], bf16, tag="zsb")
                zsb_tiles[(b, j)] = zsb
                for hh in range(2):
                    hs = slice(64 * hh, 64 * hh + 64)
                    z = ps_av.tile([65, 4, 128], f32, tag="av")
                    for t in range(nkt):
                        q0 = max(0, 128 * (t - 4 * j))
                        nc.tensor.matmul(
                            out=z[:, q0 // 128:, :], lhsT=Vs[:, b, t, hh, :],
                            rhs=E[:, t, hh, q0:],
                            start=(t == 0), stop=(t == nkt - 1))
                    d0 = den.tile([1, 4, 128], f32, tag="d0")
                    nc.vector.tensor_copy(out=d0, in_=z[64:65, :, :])
                    nc.vector.reciprocal_approx_fast(out=d0, in_=d0)
                    rb = den.tile([64, 4, 128], f32, tag="rb")
                    nc.gpsimd.partition_broadcast(out_ap=rb, in_ap=d0)
                    nc.vector.tensor_tensor(
                        out=zsb[hs, :, :], in0=z[0:64, :, :], in1=rb,
                        op=OP.mult)

            # ---- z exchange + local output projection ----
            def a2a(j):
                for b in range(2):
                    zsb = zsb_tiles.pop((b, j))
                    for s4 in range(4):
                        nc.sync.dma_start(
                            out=z_send[j].ap()[4 * b + s4],
                            in_=zsb[:, s4, :])
                nc.gpsimd.collective_compute(
                    "AllToAll", mybir.AluOpType.bypass,
                    replica_groups=GROUPS_ALL,
                    ins=[z_send[j].ap().opt()],
                    outs=[z_recv[j].ap().opt()])

            def outproj(j):
                zall = zallp.tile([128, 8, 128], bf16, tag="zall")
                for s in range(8):
                    nc.gpsimd.dma_start(out=zall[:, s, :],
                                        in_=z_recv[j].ap()[s])
                for mc in range(2):
                    po = ps_pj.tile([128, 512], f32, tag="pj")
                    for kt in range(8):
                        nc.tensor.matmul(
                            out=po, lhsT=zall[:, kt, :],
                            rhs=wo_sb[:, kt, mc * 512:(mc + 1) * 512],
                            start=(kt == 0), stop=(kt == 7))
                    o_sb = osb.tile([128, 512], f16, tag="osb")
                    nc.vector.tensor_copy(out=o_sb, in_=po)
                    nc.sync.dma_start(
                        out=out_ext[j * 128:(j + 1) * 128,
                                    mc * 512:(mc + 1) * 512],
                        in_=o_sb)

            stages = [(b, j) for j in range(NCHUNK) for b in range(2)]
            for idx, (b, j) in enumerate(stages):
                if j == 0:
                    proj_chunk(b, 0)
                scores_exp(b, j)
                if j == 0:
                    proj_v(b, 0)
                if j < NCHUNK - 1:
                    proj_chunk(b, j + 1)
                if idx >= 1:
                    pb, pj = stages[idx - 1]
                    av_normalize(pb, pj)
                    if pb == 1:
                        a2a(pj)
                if j < NCHUNK - 1:
                    proj_v(b, j + 1)
                if idx >= 3 and stages[idx - 2][0] == 1:
                    outproj(stages[idx - 2][1])
                mask(b, j)
            av_normalize(*stages[-1])
            a2a(NCHUNK - 1)
            outproj(NCHUNK - 1)

    nc.compile()
    return nc


def _get_built(with_qk_bias):
    key = bool(with_qk_bias)
    if key not in _BUILT:
        _BUILT[key] = _build(key)
    return _BUILT[key]


def _rope_tables():
    pos = np.arange(S, dtype=np.float64)
    dim = np.arange(DH // 2, dtype=np.float64)
    freq = ROT_BASE ** (dim / (DH / 2))
    freq = np.concatenate([freq, freq])                # [64]
    ang = pos[None, :] / freq[:, None]                 # [64, S]
    cos = np.cos(ang)
    sin = np.sin(ang)
    # sign of the rotate-half term folded into sin': rows 0..31 get -sin
    sinm = sin.copy()
    sinm[:DH // 2] *= -1.0
    cosT = np.tile(cos, (2, 1)).astype(BF)             # [128, S]
    sinT = np.tile(sinm, (2, 1)).astype(BF)
    return cosT, sinT


def kernel(x, W_Q, b_Q, W_K, b_K, W_V, b_V, W_O, b_O):
    from concourse.bass_utils import run_bass_kernel_spmd

    x = np.asarray(x)
    W_Q, W_K, W_V, W_O = (np.asarray(a) for a in (W_Q, W_K, W_V, W_O))
    b_Q, b_K, b_V, b_O = (np.asarray(a) for a in (b_Q, b_K, b_V, b_O))
    with_qk_bias = bool(np.any(b_Q) or np.any(b_K))
    nc = _get_built(with_qk_bias)

    cosT, sinT = _rope_tables()

    def wtile(w):            # [1024, C] -> [128, 8, C]
        c = w.shape[1]
        return np.ascontiguousarray(
            w.reshape(8, 128, c).transpose(1, 0, 2)).astype(BF)

    # x transposed per batch: [d, s]: d = kt*128 + p -> [p, kt, s]
    xT_host = np.stack([
        np.ascontiguousarray(
            x[b].T.reshape(8, 128, S).transpose(1, 0, 2)).astype(BF)
        for b in range(2)], axis=0)
    # W_O for ALL heads: slot s = heads (2s, 2s+1); identical on all cores
    wo_h = np.ascontiguousarray(
        np.concatenate([W_O[h] for h in range(NH)], axis=0)  # [1024, 1024]
        .reshape(8, 128, D).transpose(1, 0, 2)).astype(BF)

    in_maps = []
    for core in range(N_CORES):
        h0 = 2 * core
        wq_h = wtile(np.concatenate([W_Q[h0], W_Q[h0 + 1]], axis=1))
        wk_h = wtile(np.concatenate([W_K[h0], W_K[h0 + 1]], axis=1))
        wv_h = wtile(np.concatenate([W_V[h0], W_V[h0 + 1]], axis=1))
        m = {
            "xT": xT_host, "wq": wq_h, "wk": wk_h, "wv": wv_h, "wo": wo_h,
            "cosT": cosT, "sinTm": sinT,
        }
        if with_qk_bias:
            m["bq"] = np.concatenate(
                [b_Q[h0], b_Q[h0 + 1]]).astype(np.float32)[:, None]
            m["bk"] = np.concatenate(
                [b_K[h0], b_K[h0 + 1]]).astype(np.float32)[:, None]
        in_maps.append(m)

    global _last_in_maps
    _last_in_maps = in_maps
    res = run_bass_kernel_spmd(nc, in_maps, list(range(N_CORES)))

    out = np.empty((2, S, D), dtype=np.float32)
    for core in range(N_CORES):
        b, r = divmod(core, 4)
        shard = res.results[core]["out_shard"].astype(np.float32)
        for j in range(NCHUNK):
            out[b, QCHUNK * j + 128 * r: QCHUNK * j + 128 * (r + 1), :] = \
                shard[128 * j:128 * (j + 1)]

    # b_V shifts z by exactly b_V (softmax rows sum to 1); fold with b_O.
    corr = b_O.astype(np.float64).copy()
    if np.any(b_V):
        corr = corr + np.einsum("hd,hdm->m", b_V.astype(np.float64),
                                W_O.astype(np.float64))
    if np.any(corr):
        out = out + corr.astype(np.float32)
    return out


# revision 26
# speedup vs baseline: 1.0563x; 1.0563x over previous
"""Trainium2 Bass kernel for causal multi-head attention with NeoX RoPE.

Problem: x[2, 2048, 1024], 16 heads x d_head 64, rotary over all 64 dims,
causal softmax, output projection.

Sharding (v2): every core holds 2 heads ({2c, 2c+1}) and processes BOTH
batches.  The output projection is computed after a per-q-chunk 8-core
AllToAll of the normalized z shards: core c sends z[b, qsub, its 2 heads]
to the core that owns output rows (b, qsub); each core then contracts all
16 heads locally (W_O resident, slot s = heads {2s, 2s+1}) and writes its
own [128 x 1024] output rows per chunk.  This replaces the f16-partial
ReduceScatter chain of v1 (4x less collective traffic, no serial RS tail).

Per-core dataflow (batch b, 2 heads packed on 128 partitions):
  xT [b, d, s] (bf16, host-transposed)
  Q/K projections -> [128, s] via PE; RoPE as q*cos + flip(q)*sin' with the
    rotate-half flip fused into per-32-partition-block multiplies and the
    flip sign folded into the sin table.
  V projection -> Vs [s, h, 65] with a ones column (softmax denominator
    rides the AV matmul for free).
  Scores transposed: S_T[k, q] = kT.T @ qT per 128-k-tile; the two heads run
    as concurrent 64x128 PE row-tiles; exp on ScalarE (scale 1/8 folded in);
    causal mask via narrow GPSIMD affine_select on the 128x128 diagonal
    blocks only.
  AV: z[65, q] += V_aug.T @ E per k-tile; row 64 is the denominator.
  Normalize via reciprocal_approx_fast + partition_broadcast + multiply.
  AllToAll(z) per chunk, then out[q, m] = zall.T @ W_O locally.
"""

import numpy as np
import ml_dtypes

S = 2048
D = 1024
NH = 16
DH = 64
SCALE = 8.0
ROT_BASE = 10000.0
N_CORES = 8
QCHUNK = 512     # q chunk (free dim of score matmuls)
NCHUNK = S // QCHUNK
KTILE = 128
BF = ml_dtypes.bfloat16
GROUPS_ALL = [[0, 1, 2, 3, 4, 5, 6, 7]]

_BUILT = {}


def _build(with_qk_bias):
    import concourse.bass as bass
    import concourse.tile as tile
    from concourse import bacc, mybir

    f32 = mybir.dt.float32
    bf16 = mybir.dt.bfloat16
    f16 = mybir.dt.float16
    AF = mybir.ActivationFunctionType
    OP = mybir.AluOpType

    nc = bacc.Bacc("TRN2", target_bir_lowering=False, debug=False,
                   num_devices=N_CORES)

    xT = nc.dram_tensor("xT", [2, 128, 8, S], bf16, kind="ExternalInput").ap()
    wq = nc.dram_tensor("wq", [128, 8, 128], bf16, kind="ExternalInput").ap()
    wk = nc.dram_tensor("wk", [128, 8, 128], bf16, kind="ExternalInput").ap()
    wv = nc.dram_tensor("wv", [128, 8, 128], bf16, kind="ExternalInput").ap()
    wo = nc.dram_tensor("wo", [128, 8, D], bf16, kind="ExternalInput").ap()
    cosd = nc.dram_tensor("cosT", [128, S], bf16, kind="ExternalInput").ap()
    sind = nc.dram_tensor("sinTm", [128, S], bf16, kind="ExternalInput").ap()
    if with_qk_bias:
        bqd = nc.dram_tensor("bq", [128, 1], f32, kind="ExternalInput").ap()
        bkd = nc.dram_tensor("bk", [128, 1], f32, kind="ExternalInput").ap()

    z_send = [nc.dram_tensor(f"z_send{j}", [8, 128, 128], bf16)
              for j in range(NCHUNK)]
    z_recv = [nc.dram_tensor(f"z_recv{j}", [8, 128, 128], bf16)
              for j in range(NCHUNK)]
    out_ext = nc.dram_tensor("out_shard", [S // 4, D], f16,
                             kind="ExternalOutput").ap()

    with tile.TileContext(nc) as tc:
        with (
            tc.tile_pool(name="consts", bufs=1) as consts,
            tc.tile_pool(name="qk", bufs=1) as qkpool,
            tc.tile_pool(name="vsb", bufs=1) as vpool,
            tc.tile_pool(name="rope", bufs=2) as rope,
            tc.tile_pool(name="epool", bufs=2) as epool,
            tc.tile_pool(name="zpool", bufs=4) as zpool,
            tc.tile_pool(name="den", bufs=2) as den,
            tc.tile_pool(name="zail", bufs=2) as zallp,
            tc.tile_pool(name="osb", bufs=2) as osb,
            tc.tile_pool(name="ps_sc", bufs=2, space="PSUM") as ps_sc,
            tc.tile_pool(name="ps_av", bufs=2, space="PSUM") as ps_av,
            tc.tile_pool(name="ps_pj", bufs=2, space="PSUM") as ps_pj,
        ):
            wq_sb = consts.tile([128, 8, 128], bf16, tag="wq")
            nc.gpsimd.dma_start(out=wq_sb, in_=wq)
            wk_sb = consts.tile([128, 8, 128], bf16, tag="wk")
            nc.gpsimd.dma_start(out=wk_sb, in_=wk)
            cos_sb = consts.tile([128, S], bf16, tag="cos")
            nc.gpsimd.dma_start(out=cos_sb, in_=cosd)
            sin_sb = consts.tile([128, S], bf16, tag="sin")
            nc.gpsimd.dma_start(out=sin_sb, in_=sind)
            wv_sb = consts.tile([128, 8, 128], bf16, tag="wv")
            nc.gpsimd.dma_start(out=wv_sb, in_=wv)
            wo_sb = consts.tile([128, 8, D], bf16, tag="wo")
            nc.gpsimd.dma_start(out=wo_sb, in_=wo)

            xT_sb = consts.tile([128, 2, 8, S], bf16, tag="xT")
            # first-needed slices first: batch 0 cols 0:512, then the rest
            for kt in range(8):
                nc.sync.dma_start(out=xT_sb[:, 0, kt, 0:512],
                                  in_=xT[0][:, kt, 0:512])
            for kt in range(8):
                nc.sync.dma_start(out=xT_sb[:, 0, kt, 512:S],
                                  in_=xT[0][:, kt, 512:S])
            for kt in range(8):
                nc.sync.dma_start(out=xT_sb[:, 1, kt, :], in_=xT[1][:, kt, :])
            if with_qk_bias:
                bq_sb = consts.tile([128, 1], f32, tag="bq")
                nc.sync.dma_start(out=bq_sb, in_=bqd)
                bk_sb = consts.tile([128, 1], f32, tag="bk")
                nc.sync.dma_start(out=bk_sb, in_=bkd)

            warm = consts.tile([128, 8], f32, tag="warm")
            nc.vector.memset(warm, 0.0)
            nc.scalar.activation(out=warm, in_=warm, func=AF.Exp, scale=1.0)

            # Persistent rotated Q/K: [128 (=2-head pack), batch, s]
            Qr = qkpool.tile([128, 2, S], bf16, tag="Qr")
            Kr = qkpool.tile([128, 2, S], bf16, tag="Kr")
            # V with ones column: [s-part, batch, s-tile, head, 65]
            Vs = vpool.tile([128, 2, 16, 2, 65], bf16, tag="Vs")
            nc.vector.memset(Vs[:, :, :, :, 64:65], 1.0)

            # ---- per-chunk projections (interleaved with attention) ----
            def proj_chunk(b, c, which=None):
                cs = slice(c * QCHUNK, (c + 1) * QCHUNK)
                sel = ((wq_sb, "bq", Qr), (wk_sb, "bk", Kr))
                if which is not None:
                    sel = (sel[which],)
                for (wsb, bias_name, dst) in sel:
                    pt = ps_pj.tile([128, QCHUNK], f32, tag="pj")
                    for kt in range(8):
                        nc.tensor.matmul(
                            out=pt, lhsT=wsb[:, kt, :],
                            rhs=xT_sb[:, b, kt, cs],
                            start=(kt == 0), stop=(kt == 7))
                    if with_qk_bias:
                        bsb = bq_sb if bias_name == "bq" else bk_sb
                        nc.vector.tensor_scalar_add(
                            out=pt, in0=pt, scalar1=bsb[:, 0:1])
                    q_sb = rope.tile([128, QCHUNK], bf16, tag="ropeA")
                    nc.vector.tensor_copy(out=q_sb, in_=pt)
                    # q_rot = q*cos + flip(q)*sin' (sign folded into sin')
                    qf = rope.tile([128, QCHUNK], bf16, tag="ropeB")
                    for blk in range(4):
                        src = (blk ^ 1) * 32
                        nc.vector.tensor_copy(
                            out=qf[blk * 32:blk * 32 + 32, :],
                            in_=q_sb[src:src + 32, :])
                    qs = rope.tile([128, QCHUNK], bf16, tag="ropeC")
                    nc.vector.tensor_tensor(
                        out=qs, in0=qf, in1=sin_sb[:, cs], op=OP.mult)
                    qc = rope.tile([128, QCHUNK], bf16, tag="ropeB")
                    nc.vector.tensor_tensor(
                        out=qc, in0=q_sb, in1=cos_sb[:, cs], op=OP.mult)
                    nc.vector.tensor_tensor(
                        out=dst[:, b, cs], in0=qc, in1=qs, op=OP.add)

            def proj_v(b, c, half=None):
                sts = (range(4 * c, 4 * c + 4) if half is None else
                       range(4 * c + 2 * half, 4 * c + 2 * half + 2))
                for st in sts:
                    pt = ps_pj.tile([128, 2, 64], f32, tag="pj")
                    for kt in range(8):
                        nc.tensor.matmul(
                            out=pt,
                            lhsT=xT_sb[:, b, kt, st * 128:(st + 1) * 128],
                            rhs=wv_sb[:, kt, :],
                            start=(kt == 0), stop=(kt == 7))
                    nc.vector.tensor_copy(
                        out=Vs[:, b, st, :, 0:64], in_=pt)

            # ---- attention (softmax pipelined with PE) ----
            # During a stage's scores burst the PE is paced by exp on
            # ScalarE (~930ns/tile vs ~430ns/tile to produce): the two
            # score-psum buffers recycle at the exp rate.  To keep the PE
            # busy, exp-INDEPENDENT work (previous stage's AV, projections,
            # output projection) is drained from a pending queue between
            # score tiles.
            E_tiles = {}
            zsb_tiles = {}

            def scores_exp(b, j, pending):
                nkt = 4 * j + 4
                E = epool.tile([128, 16, 2, QCHUNK], bf16, tag="E")
                E_tiles[(b, j)] = E
                for t in range(nkt):
                    q0 = max(0, 128 * (t - 4 * j))
                    qs2 = slice(j * QCHUNK + q0, (j + 1) * QCHUNK)
                    sc = ps_sc.tile([128, 2, QCHUNK], f32, tag="sc")
                    for hh in range(2):
                        hs = slice(64 * hh, 64 * hh + 64)
                        nc.tensor.matmul(
                            out=sc[:, hh, q0:],
                            lhsT=Kr[hs, b, t * 128:(t + 1) * 128],
                            rhs=Qr[hs, b, qs2], start=True, stop=True)
                    nc.scalar.activation(
                        out=E[:, t, :, q0:], in_=sc[:, :, q0:],
                        func=AF.Exp, scale=1.0 / SCALE)
                    if t % 2 == 1 and pending:
                        pending.pop(0)()

            def mask(b, j):
                E = E_tiles[(b, j)]
                for dt in range(4):
                    t = 4 * j + dt
                    qb = slice(128 * dt, 128 * dt + 128)
                    nc.gpsimd.affine_select(
                        out=E[:, t, :, qb], in_=E[:, t, :, qb],
                        pattern=[[0, 2], [1, 128]], base=0,
                        channel_multiplier=-1,
                        compare_op=OP.is_ge, fill=0.0)

            def av_normalize_hh(b, j, hh):
                nkt = 4 * j + 4
                if hh == 0:
                    E = E_tiles[(b, j)]
                    zsb = zpool.tile([128, 4, 128], bf16, tag="zsb")
                    zsb_tiles[(b, j)] = zsb
                else:
                    E = E_tiles.pop((b, j))
                    zsb = zsb_tiles[(b, j)]
                hs = slice(64 * hh, 64 * hh + 64)
                z = ps_av.tile([65, 4, 128], f32, tag="av")
                for t in range(nkt):
                    q0 = max(0, 128 * (t - 4 * j))
                    nc.tensor.matmul(
                        out=z[:, q0 // 128:, :], lhsT=Vs[:, b, t, hh, :],
                        rhs=E[:, t, hh, q0:],
                        start=(t == 0), stop=(t == nkt - 1))
                d0 = den.tile([1, 4, 128], f32, tag="d0")
                nc.vector.tensor_copy(out=d0, in_=z[64:65, :, :])
                nc.vector.reciprocal_approx_fast(out=d0, in_=d0)
                rb = den.tile([64, 4, 128], f32, tag="rb")
                nc.gpsimd.partition_broadcast(out_ap=rb, in_ap=d0)
                nc.vector.tensor_tensor(
                    out=zsb[hs, :, :], in0=z[0:64, :, :], in1=rb,
                    op=OP.mult)

            # ---- z exchange + local output projection ----
            def a2a(j):
                for b in range(2):
                    zsb = zsb_tiles.pop((b, j))
                    for s4 in range(4):
                        nc.sync.dma_start(
                            out=z_send[j].ap()[4 * b + s4],
                            in_=zsb[:, s4, :])
                nc.gpsimd.collective_compute(
                    "AllToAll", mybir.AluOpType.bypass,
                    replica_groups=GROUPS_ALL,
                    ins=[z_send[j].ap().opt()],
                    outs=[z_recv[j].ap().opt()])

            def outproj(j):
                zall = zallp.tile([128, 8, 128], bf16, tag="zall")
                for s in range(8):
                    nc.gpsimd.dma_start(out=zall[:, s, :],
                                        in_=z_recv[j].ap()[s])
                for mc in range(2):
                    po = ps_pj.tile([128, 512], f32, tag="pj")
                    for kt in range(8):
                        nc.tensor.matmul(
                            out=po, lhsT=zall[:, kt, :],
                            rhs=wo_sb[:, kt, mc * 512:(mc + 1) * 512],
                            start=(kt == 0), stop=(kt == 7))
                    o_sb = osb.tile([128, 512], f16, tag="osb")
                    nc.vector.tensor_copy(out=o_sb, in_=po)
                    nc.sync.dma_start(
                        out=out_ext[j * 128:(j + 1) * 128,
                                    mc * 512:(mc + 1) * 512],
                        in_=o_sb)

            stages = [(b, j) for j in range(NCHUNK) for b in range(2)]
            for idx, (b, j) in enumerate(stages):
                pending = []
                if idx >= 1:
                    pb, pj = stages[idx - 1]
                    pending.append(
                        lambda pb=pb, pj=pj: av_normalize_hh(pb, pj, 0))
                    pending.append(
                        lambda pb=pb, pj=pj: av_normalize_hh(pb, pj, 1))
                    if pb == 1:
                        pending.append(lambda pj=pj: a2a(pj))
                if j < NCHUNK - 1:
                    pending.append(
                        lambda b=b, j=j: proj_chunk(b, j + 1, which=0))
                    pending.append(
                        lambda b=b, j=j: proj_chunk(b, j + 1, which=1))
                    pending.append(
                        lambda b=b, j=j: proj_v(b, j + 1, half=0))
                    pending.append(
                        lambda b=b, j=j: proj_v(b, j + 1, half=1))
                if idx >= 3 and stages[idx - 2][0] == 1:
                    pending.append(
                        lambda jj=stages[idx - 2][1]: outproj(jj))
                if j == 0:
                    proj_chunk(b, 0)
                    proj_v(b, 0)
                scores_exp(b, j, pending)
                for th in pending:
                    th()
                mask(b, j)
            av_normalize_hh(1, NCHUNK - 1, 0)
            av_normalize_hh(1, NCHUNK - 1, 1)
            a2a(NCHUNK - 1)
            outproj(NCHUNK - 1)

    nc.compile()
    return nc


def _get_built(with_qk_bias):
    key = bool(with_qk_bias)
    if key not in _BUILT:
        _BUILT[key] = _build(key)
    return _BUILT[key]


def _rope_tables():
    pos = np.arange(S, dtype=np.float64)
    dim = np.arange(DH // 2, dtype=np.float64)
    freq = ROT_BASE ** (dim / (DH / 2))
    freq = np.concatenate([freq, freq])                # [64]
    ang = pos[None, :] / freq[:, None]                 # [64, S]
    cos = np.cos(ang)
    sin = np.sin(ang)
    # sign of the rotate-half term folded into sin': rows 0..31 get -sin
    sinm = sin.copy()
    sinm[:DH // 2] *= -1.0
    cosT = np.tile(cos, (2, 1)).astype(BF)             # [128, S]
    sinT = np.tile(sinm, (2, 1)).astype(BF)
    return cosT, sinT


def kernel(x, W_Q, b_Q, W_K, b_K, W_V, b_V, W_O, b_O):
    from concourse.bass_utils import run_bass_kernel_spmd

    x = np.asarray(x)
    W_Q, W_K, W_V, W_O = (np.asarray(a) for a in (W_Q, W_K, W_V, W_O))
    b_Q, b_K, b_V, b_O = (np.asarray(a) for a in (b_Q, b_K, b_V, b_O))
    with_qk_bias = bool(np.any(b_Q) or np.any(b_K))
    nc = _get_built(with_qk_bias)

    cosT, sinT = _rope_tables()

    def wtile(w):            # [1024, C] -> [128, 8, C]
        c = w.shape[1]
        return np.ascontiguousarray(
            w.reshape(8, 128, c).transpose(1, 0, 2)).astype(BF)

    # x transposed per batch: [d, s]: d = kt*128 + p -> [p, kt, s]
    xT_host = np.stack([
        np.ascontiguousarray(
            x[b].T.reshape(8, 128, S).transpose(1, 0, 2)).astype(BF)
        for b in range(2)], axis=0)
    # W_O for ALL heads: slot s = heads (2s, 2s+1); identical on all cores
    wo_h = np.ascontiguousarray(
        np.concatenate([W_O[h] for h in range(NH)], axis=0)  # [1024, 1024]
        .reshape(8, 128, D).transpose(1, 0, 2)).astype(BF)

    in_maps = []
    for core in range(N_CORES):
        h0 = 2 * core
        wq_h = wtile(np.concatenate([W_Q[h0], W_Q[h0 + 1]], axis=1))
        wk_h = wtile(np.concatenate([W_K[h0], W_K[h0 + 1]], axis=1))
        wv_h = wtile(np.concatenate([W_V[h0], W_V[h0 + 1]], axis=1))
        m = {
            "xT": xT_host, "wq": wq_h, "wk": wk_h, "wv": wv_h, "wo": wo_h,
            "cosT": cosT, "sinTm": sinT,
        }
        if with_qk_bias:
            m["bq"] = np.concatenate(
                [b_Q[h0], b_Q[h0 + 1]]).astype(np.float32)[:, None]
            m["bk"] = np.concatenate(
                [b_K[h0], b_K[h0 + 1]]).astype(np.float32)[:, None]
        in_maps.append(m)

    global _last_in_maps
    _last_in_maps = in_maps
    res = run_bass_kernel_spmd(nc, in_maps, list(range(N_CORES)))

    out = np.empty((2, S, D), dtype=np.float32)
    for core in range(N_CORES):
        b, r = divmod(core, 4)
        shard = res.results[core]["out_shard"].astype(np.float32)
        for j in range(NCHUNK):
            out[b, QCHUNK * j + 128 * r: QCHUNK * j + 128 * (r + 1), :] = \
                shard[128 * j:128 * (j + 1)]

    # b_V shifts z by exactly b_V (softmax rows sum to 1); fold with b_O.
    corr = b_O.astype(np.float64).copy()
    if np.any(b_V):
        corr = corr + np.einsum("hd,hdm->m", b_V.astype(np.float64),
                                W_O.astype(np.float64))
    if np.any(corr):
        out = out + corr.astype(np.float32)
    return out


# revision 27
# speedup vs baseline: 1.1364x; 1.0759x over previous
"""Trainium2 Bass kernel for causal multi-head attention with NeoX RoPE.

Problem: x[2, 2048, 1024], 16 heads x d_head 64, rotary over all 64 dims,
causal softmax, output projection.

Sharding (v2): every core holds 2 heads ({2c, 2c+1}) and processes BOTH
batches.  The output projection is computed after a per-q-chunk 8-core
AllToAll of the normalized z shards: core c sends z[b, qsub, its 2 heads]
to the core that owns output rows (b, qsub); each core then contracts all
16 heads locally (W_O resident, slot s = heads {2s, 2s+1}) and writes its
own [128 x 1024] output rows per chunk.  This replaces the f16-partial
ReduceScatter chain of v1 (4x less collective traffic, no serial RS tail).

Per-core dataflow (batch b, 2 heads packed on 128 partitions):
  xT [b, d, s] (bf16, host-transposed)
  Q/K projections -> [128, s] via PE; RoPE as q*cos + flip(q)*sin' with the
    rotate-half flip fused into per-32-partition-block multiplies and the
    flip sign folded into the sin table.
  V projection -> Vs [s, h, 65] with a ones column (softmax denominator
    rides the AV matmul for free).
  Scores transposed: S_T[k, q] = kT.T @ qT per 128-k-tile; the two heads run
    as concurrent 64x128 PE row-tiles; exp on ScalarE (scale 1/8 folded in);
    causal mask via narrow GPSIMD affine_select on the 128x128 diagonal
    blocks only.
  AV: z[65, q] += V_aug.T @ E per k-tile; row 64 is the denominator.
  Normalize via reciprocal_approx_fast + partition_broadcast + multiply.
  AllToAll(z) per chunk, then out[q, m] = zall.T @ W_O locally.
"""

import numpy as np
import ml_dtypes

S = 2048
D = 1024
NH = 16
DH = 64
SCALE = 8.0
ROT_BASE = 10000.0
N_CORES = 8
QCHUNK = 512     # q chunk (free dim of score matmuls)
NCHUNK = S // QCHUNK
KTILE = 128
BF = ml_dtypes.bfloat16
GROUPS_ALL = [[0, 1, 2, 3, 4, 5, 6, 7]]

_BUILT = {}


def _build(with_qk_bias):
    import concourse.bass as bass
    import concourse.tile as tile
    from concourse import bacc, mybir

    f32 = mybir.dt.float32
    bf16 = mybir.dt.bfloat16
    f16 = mybir.dt.float16
    AF = mybir.ActivationFunctionType
    OP = mybir.AluOpType

    nc = bacc.Bacc("TRN2", target_bir_lowering=False, debug=False,
                   num_devices=N_CORES)

    xT = nc.dram_tensor("xT", [2, 128, 8, S], bf16, kind="ExternalInput").ap()
    wq = nc.dram_tensor("wq", [128, 8, 128], bf16, kind="ExternalInput").ap()
    wk = nc.dram_tensor("wk", [128, 8, 128], bf16, kind="ExternalInput").ap()
    wv = nc.dram_tensor("wv", [128, 8, 128], bf16, kind="ExternalInput").ap()
    wo = nc.dram_tensor("wo", [128, 8, D], bf16, kind="ExternalInput").ap()
    cosd = nc.dram_tensor("cosT", [128, S], bf16, kind="ExternalInput").ap()
    sind = nc.dram_tensor("sinTm", [128, S], bf16, kind="ExternalInput").ap()
    if with_qk_bias:
        bqd = nc.dram_tensor("bq", [128, 1], f32, kind="ExternalInput").ap()
        bkd = nc.dram_tensor("bk", [128, 1], f32, kind="ExternalInput").ap()

    z_send = [nc.dram_tensor(f"z_send{j}", [8, 128, 128], bf16)
              for j in range(NCHUNK)]
    z_recv = [nc.dram_tensor(f"z_recv{j}", [8, 128, 128], bf16)
              for j in range(NCHUNK)]
    out_ext = nc.dram_tensor("out_shard", [S // 4, D], f16,
                             kind="ExternalOutput").ap()

    with tile.TileContext(nc) as tc:
        with (
            tc.tile_pool(name="consts", bufs=1) as consts,
            tc.tile_pool(name="qk", bufs=1) as qkpool,
            tc.tile_pool(name="vsb", bufs=1) as vpool,
            tc.tile_pool(name="rope", bufs=2) as rope,
            tc.tile_pool(name="epool", bufs=2) as epool,
            tc.tile_pool(name="zpool", bufs=4) as zpool,
            tc.tile_pool(name="den", bufs=2) as den,
            tc.tile_pool(name="zail", bufs=2) as zallp,
            tc.tile_pool(name="osb", bufs=2) as osb,
            tc.tile_pool(name="ps_sc", bufs=2, space="PSUM") as ps_sc,
            tc.tile_pool(name="ps_av", bufs=2, space="PSUM") as ps_av,
            tc.tile_pool(name="ps_pj", bufs=2, space="PSUM") as ps_pj,
        ):
            wq_sb = consts.tile([128, 8, 128], bf16, tag="wq")
            nc.gpsimd.dma_start(out=wq_sb, in_=wq)
            wk_sb = consts.tile([128, 8, 128], bf16, tag="wk")
            nc.gpsimd.dma_start(out=wk_sb, in_=wk)
            cos_sb = consts.tile([128, S], bf16, tag="cos")
            nc.gpsimd.dma_start(out=cos_sb, in_=cosd)
            sin_sb = consts.tile([128, S], bf16, tag="sin")
            nc.gpsimd.dma_start(out=sin_sb, in_=sind)
            wv_sb = consts.tile([128, 8, 128], bf16, tag="wv")
            nc.gpsimd.dma_start(out=wv_sb, in_=wv)
            wo_sb = consts.tile([128, 8, D], bf16, tag="wo")
            nc.gpsimd.dma_start(out=wo_sb, in_=wo)

            xT_sb = consts.tile([128, 2, 8, S], bf16, tag="xT")
            # first-needed slices first: batch 0 cols 0:512, then the rest
            for kt in range(8):
                nc.sync.dma_start(out=xT_sb[:, 0, kt, 0:512],
                                  in_=xT[0][:, kt, 0:512])
            for kt in range(8):
                nc.sync.dma_start(out=xT_sb[:, 0, kt, 512:S],
                                  in_=xT[0][:, kt, 512:S])
            for kt in range(8):
                nc.sync.dma_start(out=xT_sb[:, 1, kt, :], in_=xT[1][:, kt, :])
            if with_qk_bias:
                bq_sb = consts.tile([128, 1], f32, tag="bq")
                nc.sync.dma_start(out=bq_sb, in_=bqd)
                bk_sb = consts.tile([128, 1], f32, tag="bk")
                nc.sync.dma_start(out=bk_sb, in_=bkd)

            warm = consts.tile([128, 8], f32, tag="warm")
            nc.vector.memset(warm, 0.0)
            nc.scalar.activation(out=warm, in_=warm, func=AF.Exp, scale=1.0)

            # Persistent rotated Q/K: [128 (=2-head pack), batch, s]
            Qr = qkpool.tile([128, 2, S], bf16, tag="Qr")
            Kr = qkpool.tile([128, 2, S], bf16, tag="Kr")
            # V with ones column: [s-part, batch, s-tile, head, 65]
            Vs = vpool.tile([128, 2, 16, 2, 65], bf16, tag="Vs")
            nc.vector.memset(Vs[:, :, :, :, 64:65], 1.0)

            # ---- per-chunk projections (interleaved with attention) ----
            def proj_chunk(b, c, which=None):
                cs = slice(c * QCHUNK, (c + 1) * QCHUNK)
                sel = ((wq_sb, "bq", Qr), (wk_sb, "bk", Kr))
                if which is not None:
                    sel = (sel[which],)
                for (wsb, bias_name, dst) in sel:
                    pt = ps_pj.tile([128, QCHUNK], f32, tag="pj")
                    for kt in range(8):
                        nc.tensor.matmul(
                            out=pt, lhsT=wsb[:, kt, :],
                            rhs=xT_sb[:, b, kt, cs],
                            start=(kt == 0), stop=(kt == 7))
                    if with_qk_bias:
                        bsb = bq_sb if bias_name == "bq" else bk_sb
                        nc.vector.tensor_scalar_add(
                            out=pt, in0=pt, scalar1=bsb[:, 0:1])
                    q_sb = rope.tile([128, QCHUNK], bf16, tag="ropeA")
                    nc.vector.tensor_copy(out=q_sb, in_=pt)
                    # q_rot = q*cos + flip(q)*sin' (sign folded into sin')
                    qf = rope.tile([128, QCHUNK], bf16, tag="ropeB")
                    for blk in range(4):
                        src = (blk ^ 1) * 32
                        nc.vector.tensor_copy(
                            out=qf[blk * 32:blk * 32 + 32, :],
                            in_=q_sb[src:src + 32, :])
                    qs = rope.tile([128, QCHUNK], bf16, tag="ropeC")
                    nc.vector.tensor_tensor(
                        out=qs, in0=qf, in1=sin_sb[:, cs], op=OP.mult)
                    qc = rope.tile([128, QCHUNK], bf16, tag="ropeB")
                    nc.vector.tensor_tensor(
                        out=qc, in0=q_sb, in1=cos_sb[:, cs], op=OP.mult)
                    nc.vector.tensor_tensor(
                        out=dst[:, b, cs], in0=qc, in1=qs, op=OP.add)

            def proj_v(b, c, half=None):
                sts = (range(4 * c, 4 * c + 4) if half is None else
                       range(4 * c + 2 * half, 4 * c + 2 * half + 2))
                for st in sts:
                    pt = ps_pj.tile([128, 2, 64], f32, tag="pj")
                    for kt in range(8):
                        nc.tensor.matmul(
                            out=pt,
                            lhsT=xT_sb[:, b, kt, st * 128:(st + 1) * 128],
                            rhs=wv_sb[:, kt, :],
                            start=(kt == 0), stop=(kt == 7))
                    nc.vector.tensor_copy(
                        out=Vs[:, b, st, :, 0:64], in_=pt)

            # ---- attention (softmax pipelined with PE) ----
            # During a stage's scores burst the PE is paced by exp on
            # ScalarE (~930ns/tile vs ~430ns/tile to produce): the two
            # score-psum buffers recycle at the exp rate.  To keep the PE
            # busy, exp-INDEPENDENT work (previous stage's AV, projections,
            # output projection) is drained from a pending queue between
            # score tiles.
            E_tiles = {}
            zsb_tiles = {}

            def scores_exp(b, j, pending):
                nkt = 4 * j + 4
                E = epool.tile([128, 16, 2, QCHUNK], bf16, tag="E")
                E_tiles[(b, j)] = E
                for t in range(nkt):
                    q0 = max(0, 128 * (t - 4 * j))
                    qs2 = slice(j * QCHUNK + q0, (j + 1) * QCHUNK)
                    sc = ps_sc.tile([128, 2, QCHUNK], f32, tag="sc")
                    for hh in range(2):
                        hs = slice(64 * hh, 64 * hh + 64)
                        nc.tensor.matmul(
                            out=sc[:, hh, q0:],
                            lhsT=Kr[hs, b, t * 128:(t + 1) * 128],
                            rhs=Qr[hs, b, qs2], start=True, stop=True)
                    nc.scalar.activation(
                        out=E[:, t, :, q0:], in_=sc[:, :, q0:],
                        func=AF.Exp, scale=1.0 / SCALE)
                    if t % 2 == 1 and pending:
                        pending.pop(0)()

            def mask(b, j):
                E = E_tiles[(b, j)]
                for dt in range(4):
                    t = 4 * j + dt
                    qb = slice(128 * dt, 128 * dt + 128)
                    nc.gpsimd.affine_select(
                        out=E[:, t, :, qb], in_=E[:, t, :, qb],
                        pattern=[[0, 2], [1, 128]], base=0,
                        channel_multiplier=-1,
                        compare_op=OP.is_ge, fill=0.0)

            def av_normalize_hh(b, j, hh):
                nkt = 4 * j + 4
                if hh == 0:
                    E = E_tiles[(b, j)]
                    zsb = zpool.tile([128, 4, 128], bf16, tag="zsb")
                    zsb_tiles[(b, j)] = zsb
                else:
                    E = E_tiles.pop((b, j))
                    zsb = zsb_tiles[(b, j)]
                hs = slice(64 * hh, 64 * hh + 64)
                z = ps_av.tile([65, 4, 128], f32, tag="av")
                for t in range(nkt):
                    q0 = max(0, 128 * (t - 4 * j))
                    nc.tensor.matmul(
                        out=z[:, q0 // 128:, :], lhsT=Vs[:, b, t, hh, :],
                        rhs=E[:, t, hh, q0:],
                        start=(t == 0), stop=(t == nkt - 1))
                d0 = den.tile([1, 4, 128], f32, tag="d0")
                nc.vector.tensor_copy(out=d0, in_=z[64:65, :, :])
                nc.vector.reciprocal_approx_fast(out=d0, in_=d0)
                rb = den.tile([64, 4, 128], f32, tag="rb")
                nc.gpsimd.partition_broadcast(out_ap=rb, in_ap=d0)
                nc.vector.tensor_tensor(
                    out=zsb[hs, :, :], in0=z[0:64, :, :], in1=rb,
                    op=OP.mult)

            # ---- z exchange + local output projection ----
            def a2a(j):
                for b in range(2):
                    zsb = zsb_tiles.pop((b, j))
                    for s4 in range(4):
                        nc.sync.dma_start(
                            out=z_send[j].ap()[4 * b + s4],
                            in_=zsb[:, s4, :])
                nc.gpsimd.collective_compute(
                    "AllToAll", mybir.AluOpType.bypass,
                    replica_groups=GROUPS_ALL,
                    ins=[z_send[j].ap().opt()],
                    outs=[z_recv[j].ap().opt()])

            def outproj(j):
                zall = zallp.tile([128, 8, 128], bf16, tag="zall")
                for s in range(8):
                    nc.gpsimd.dma_start(out=zall[:, s, :],
                                        in_=z_recv[j].ap()[s])
                for mc in range(2):
                    po = ps_pj.tile([128, 512], f32, tag="pj")
                    for kt in range(8):
                        nc.tensor.matmul(
                            out=po, lhsT=zall[:, kt, :],
                            rhs=wo_sb[:, kt, mc * 512:(mc + 1) * 512],
                            start=(kt == 0), stop=(kt == 7))
                    o_sb = osb.tile([128, 512], f16, tag="osb")
                    nc.vector.tensor_copy(out=o_sb, in_=po)
                    nc.sync.dma_start(
                        out=out_ext[j * 128:(j + 1) * 128,
                                    mc * 512:(mc + 1) * 512],
                        in_=o_sb)

            stages = [(b, j) for j in range(NCHUNK) for b in range(2)]
            for idx, (b, j) in enumerate(stages):
                pending = []
                if idx >= 1:
                    pb, pj = stages[idx - 1]
                    pending.append(
                        lambda pb=pb, pj=pj: av_normalize_hh(pb, pj, 0))
                    pending.append(
                        lambda pb=pb, pj=pj: av_normalize_hh(pb, pj, 1))
                    if pb == 1:
                        pending.append(lambda pj=pj: a2a(pj))
                if j < NCHUNK - 1:
                    pending.append(
                        lambda b=b, j=j: proj_chunk(b, j + 1, which=0))
                    pending.append(
                        lambda b=b, j=j: proj_chunk(b, j + 1, which=1))
                    pending.append(
                        lambda b=b, j=j: proj_v(b, j + 1, half=0))
                    pending.append(
                        lambda b=b, j=j: proj_v(b, j + 1, half=1))
                if j == 0:
                    proj_chunk(b, 0)
                    proj_v(b, 0)
                scores_exp(b, j, pending)
                for th in pending:
                    th()
                if idx >= 3 and stages[idx - 2][0] == 1:
                    outproj(stages[idx - 2][1])
                mask(b, j)
            av_normalize_hh(1, NCHUNK - 1, 0)
            av_normalize_hh(1, NCHUNK - 1, 1)
            a2a(NCHUNK - 1)
            outproj(NCHUNK - 1)

    nc.compile()
    return nc


def _get_built(with_qk_bias):
    key = bool(with_qk_bias)
    if key not in _BUILT:
        _BUILT[key] = _build(key)
    return _BUILT[key]


def _rope_tables():
    pos = np.arange(S, dtype=np.float64)
    dim = np.arange(DH // 2, dtype=np.float64)
    freq = ROT_BASE ** (dim / (DH / 2))
    freq = np.concatenate([freq, freq])                # [64]
    ang = pos[None, :] / freq[:, None]                 # [64, S]
    cos = np.cos(ang)
    sin = np.sin(ang)
    # sign of the rotate-half term folded into sin': rows 0..31 get -sin
    sinm = sin.copy()
    sinm[:DH // 2] *= -1.0
    cosT = np.tile(cos, (2, 1)).astype(BF)             # [128, S]
    sinT = np.tile(sinm, (2, 1)).astype(BF)
    return cosT, sinT


def kernel(x, W_Q, b_Q, W_K, b_K, W_V, b_V, W_O, b_O):
    from concourse.bass_utils import run_bass_kernel_spmd

    x = np.asarray(x)
    W_Q, W_K, W_V, W_O = (np.asarray(a) for a in (W_Q, W_K, W_V, W_O))
    b_Q, b_K, b_V, b_O = (np.asarray(a) for a in (b_Q, b_K, b_V, b_O))
    with_qk_bias = bool(np.any(b_Q) or np.any(b_K))
    nc = _get_built(with_qk_bias)

    cosT, sinT = _rope_tables()

    def wtile(w):            # [1024, C] -> [128, 8, C]
        c = w.shape[1]
        return np.ascontiguousarray(
            w.reshape(8, 128, c).transpose(1, 0, 2)).astype(BF)

    # x transposed per batch: [d, s]: d = kt*128 + p -> [p, kt, s]
    xT_host = np.stack([
        np.ascontiguousarray(
            x[b].T.reshape(8, 128, S).transpose(1, 0, 2)).astype(BF)
        for b in range(2)], axis=0)
    # W_O for ALL heads: slot s = heads (2s, 2s+1); identical on all cores
    wo_h = np.ascontiguousarray(
        np.concatenate([W_O[h] for h in range(NH)], axis=0)  # [1024, 1024]
        .reshape(8, 128, D).transpose(1, 0, 2)).astype(BF)

    in_maps = []
    for core in range(N_CORES):
        h0 = 2 * core
        wq_h = wtile(np.concatenate([W_Q[h0], W_Q[h0 + 1]], axis=1))
        wk_h = wtile(np.concatenate([W_K[h0], W_K[h0 + 1]], axis=1))
        wv_h = wtile(np.concatenate([W_V[h0], W_V[h0 + 1]], axis=1))
        m = {
            "xT": xT_host, "wq": wq_h, "wk": wk_h, "wv": wv_h, "wo": wo_h,
            "cosT": cosT, "sinTm": sinT,
        }
        if with_qk_bias:
            m["bq"] = np.concatenate(
                [b_Q[h0], b_Q[h0 + 1]]).astype(np.float32)[:, None]
            m["bk"] = np.concatenate(
                [b_K[h0], b_K[h0 + 1]]).astype(np.float32)[:, None]
        in_maps.append(m)

    global _last_in_maps
    _last_in_maps = in_maps
    res = run_bass_kernel_spmd(nc, in_maps, list(range(N_CORES)))

    out = np.empty((2, S, D), dtype=np.float32)
    for core in range(N_CORES):
        b, r = divmod(core, 4)
        shard = res.results[core]["out_shard"].astype(np.float32)
        for j in range(NCHUNK):
            out[b, QCHUNK * j + 128 * r: QCHUNK * j + 128 * (r + 1), :] = \
                shard[128 * j:128 * (j + 1)]

    # b_V shifts z by exactly b_V (softmax rows sum to 1); fold with b_O.
    corr = b_O.astype(np.float64).copy()
    if np.any(b_V):
        corr = corr + np.einsum("hd,hdm->m", b_V.astype(np.float64),
                                W_O.astype(np.float64))
    if np.any(corr):
        out = out + corr.astype(np.float32)
    return out


# revision 28
# speedup vs baseline: 1.1910x; 1.0480x over previous
"""Trainium2 Bass kernel for causal multi-head attention with NeoX RoPE.

Problem: x[2, 2048, 1024], 16 heads x d_head 64, rotary over all 64 dims,
causal softmax, output projection.

Sharding (v2): every core holds 2 heads ({2c, 2c+1}) and processes BOTH
batches.  The output projection is computed after a per-q-chunk 8-core
AllToAll of the normalized z shards: core c sends z[b, qsub, its 2 heads]
to the core that owns output rows (b, qsub); each core then contracts all
16 heads locally (W_O resident, slot s = heads {2s, 2s+1}) and writes its
own [128 x 1024] output rows per chunk.  This replaces the f16-partial
ReduceScatter chain of v1 (4x less collective traffic, no serial RS tail).

Per-core dataflow (batch b, 2 heads packed on 128 partitions):
  xT [b, d, s] (bf16, host-transposed)
  Q/K projections -> [128, s] via PE; RoPE as q*cos + flip(q)*sin' with the
    rotate-half flip fused into per-32-partition-block multiplies and the
    flip sign folded into the sin table.
  V projection -> Vs [s, h, 65] with a ones column (softmax denominator
    rides the AV matmul for free).
  Scores transposed: S_T[k, q] = kT.T @ qT per 128-k-tile; the two heads run
    as concurrent 64x128 PE row-tiles; exp on ScalarE (scale 1/8 folded in);
    causal mask via narrow GPSIMD affine_select on the 128x128 diagonal
    blocks only.
  AV: z[65, q] += V_aug.T @ E per k-tile; row 64 is the denominator.
  Normalize via reciprocal_approx_fast + partition_broadcast + multiply.
  AllToAll(z) per chunk, then out[q, m] = zall.T @ W_O locally.
"""

import numpy as np
import ml_dtypes

S = 2048
D = 1024
NH = 16
DH = 64
SCALE = 8.0
ROT_BASE = 10000.0
N_CORES = 8
QCHUNK = 512     # q chunk (free dim of score matmuls)
NCHUNK = S // QCHUNK
KTILE = 128
BF = ml_dtypes.bfloat16
GROUPS_ALL = [[0, 1, 2, 3, 4, 5, 6, 7]]

_BUILT = {}


def _build(with_qk_bias):
    import concourse.bass as bass
    import concourse.tile as tile
    from concourse import bacc, mybir

    f32 = mybir.dt.float32
    bf16 = mybir.dt.bfloat16
    f16 = mybir.dt.float16
    AF = mybir.ActivationFunctionType
    OP = mybir.AluOpType

    nc = bacc.Bacc("TRN2", target_bir_lowering=False, debug=False,
                   num_devices=N_CORES)

    xT = nc.dram_tensor("xT", [2, 128, 8, S], bf16, kind="ExternalInput").ap()
    wq = nc.dram_tensor("wq", [128, 8, 128], bf16, kind="ExternalInput").ap()
    wk = nc.dram_tensor("wk", [128, 8, 128], bf16, kind="ExternalInput").ap()
    wv = nc.dram_tensor("wv", [128, 8, 128], bf16, kind="ExternalInput").ap()
    wo = nc.dram_tensor("wo", [128, 8, D], bf16, kind="ExternalInput").ap()
    cosd = nc.dram_tensor("cosT", [128, S], bf16, kind="ExternalInput").ap()
    sind = nc.dram_tensor("sinTm", [128, S], bf16, kind="ExternalInput").ap()
    if with_qk_bias:
        bqd = nc.dram_tensor("bq", [128, 1], f32, kind="ExternalInput").ap()
        bkd = nc.dram_tensor("bk", [128, 1], f32, kind="ExternalInput").ap()

    z_send = [nc.dram_tensor(f"z_send{j}", [8, 128, 128], bf16)
              for j in range(NCHUNK)]
    z_recv = [nc.dram_tensor(f"z_recv{j}", [8, 128, 128], bf16)
              for j in range(NCHUNK)]
    out_ext = nc.dram_tensor("out_shard", [S // 4, D], f16,
                             kind="ExternalOutput").ap()

    with tile.TileContext(nc) as tc:
        with (
            tc.tile_pool(name="consts", bufs=1) as consts,
            tc.tile_pool(name="qk", bufs=1) as qkpool,
            tc.tile_pool(name="vsb", bufs=1) as vpool,
            tc.tile_pool(name="rope", bufs=2) as rope,
            tc.tile_pool(name="epool", bufs=2) as epool,
            tc.tile_pool(name="zpool", bufs=4) as zpool,
            tc.tile_pool(name="den", bufs=2) as den,
            tc.tile_pool(name="zail", bufs=2) as zallp,
            tc.tile_pool(name="osb", bufs=2) as osb,
            tc.tile_pool(name="ps_sc", bufs=2, space="PSUM") as ps_sc,
            tc.tile_pool(name="ps_av", bufs=2, space="PSUM") as ps_av,
            tc.tile_pool(name="ps_pj", bufs=2, space="PSUM") as ps_pj,
        ):
            wq_sb = consts.tile([128, 8, 128], bf16, tag="wq")
            nc.gpsimd.dma_start(out=wq_sb, in_=wq)
            wk_sb = consts.tile([128, 8, 128], bf16, tag="wk")
            nc.gpsimd.dma_start(out=wk_sb, in_=wk)
            cos_sb = consts.tile([128, S], bf16, tag="cos")
            nc.gpsimd.dma_start(out=cos_sb, in_=cosd)
            sin_sb = consts.tile([128, S], bf16, tag="sin")
            nc.gpsimd.dma_start(out=sin_sb, in_=sind)
            wv_sb = consts.tile([128, 8, 128], bf16, tag="wv")
            nc.gpsimd.dma_start(out=wv_sb, in_=wv)
            wo_sb = consts.tile([128, 8, D], bf16, tag="wo")
            nc.gpsimd.dma_start(out=wo_sb, in_=wo)

            xT_sb = consts.tile([128, 2, 8, S], bf16, tag="xT")
            # first-needed slices first: batch 0 cols 0:512, then the rest
            for kt in range(8):
                nc.sync.dma_start(out=xT_sb[:, 0, kt, 0:512],
                                  in_=xT[0][:, kt, 0:512])
            for kt in range(8):
                nc.sync.dma_start(out=xT_sb[:, 0, kt, 512:S],
                                  in_=xT[0][:, kt, 512:S])
            for kt in range(8):
                nc.sync.dma_start(out=xT_sb[:, 1, kt, :], in_=xT[1][:, kt, :])
            if with_qk_bias:
                bq_sb = consts.tile([128, 1], f32, tag="bq")
                nc.sync.dma_start(out=bq_sb, in_=bqd)
                bk_sb = consts.tile([128, 1], f32, tag="bk")
                nc.sync.dma_start(out=bk_sb, in_=bkd)

            warm = consts.tile([128, 8], f32, tag="warm")
            nc.vector.memset(warm, 0.0)
            nc.scalar.activation(out=warm, in_=warm, func=AF.Exp, scale=1.0)

            # Persistent rotated Q/K: [128 (=2-head pack), batch, s]
            Qr = qkpool.tile([128, 2, S], bf16, tag="Qr")
            Kr = qkpool.tile([128, 2, S], bf16, tag="Kr")
            # V with ones column: [s-part, batch, s-tile, head, 65]
            Vs = vpool.tile([128, 2, 16, 2, 65], bf16, tag="Vs")
            nc.vector.memset(Vs[:, :, :, :, 64:65], 1.0)

            # ---- per-chunk projections (interleaved with attention) ----
            def proj_chunk(b, c, which=None):
                cs = slice(c * QCHUNK, (c + 1) * QCHUNK)
                sel = ((wq_sb, "bq", Qr), (wk_sb, "bk", Kr))
                if which is not None:
                    sel = (sel[which],)
                for (wsb, bias_name, dst) in sel:
                    pt = ps_pj.tile([128, QCHUNK], f32, tag="pj")
                    for kt in range(8):
                        nc.tensor.matmul(
                            out=pt, lhsT=wsb[:, kt, :],
                            rhs=xT_sb[:, b, kt, cs],
                            start=(kt == 0), stop=(kt == 7))
                    if with_qk_bias:
                        bsb = bq_sb if bias_name == "bq" else bk_sb
                        nc.vector.tensor_scalar_add(
                            out=pt, in0=pt, scalar1=bsb[:, 0:1])
                    q_sb = rope.tile([128, QCHUNK], bf16, tag="ropeA")
                    nc.vector.tensor_copy(out=q_sb, in_=pt)
                    # q_rot = q*cos + flip(q)*sin' (sign folded into sin')
                    qf = rope.tile([128, QCHUNK], bf16, tag="ropeB")
                    for blk in range(4):
                        src = (blk ^ 1) * 32
                        nc.vector.tensor_copy(
                            out=qf[blk * 32:blk * 32 + 32, :],
                            in_=q_sb[src:src + 32, :])
                    qs = rope.tile([128, QCHUNK], bf16, tag="ropeC")
                    nc.vector.tensor_tensor(
                        out=qs, in0=qf, in1=sin_sb[:, cs], op=OP.mult)
                    qc = rope.tile([128, QCHUNK], bf16, tag="ropeB")
                    nc.vector.tensor_tensor(
                        out=qc, in0=q_sb, in1=cos_sb[:, cs], op=OP.mult)
                    nc.vector.tensor_tensor(
                        out=dst[:, b, cs], in0=qc, in1=qs, op=OP.add)

            def proj_v(b, c, half=None):
                sts = (range(4 * c, 4 * c + 4) if half is None else
                       range(4 * c + 2 * half, 4 * c + 2 * half + 2))
                for st in sts:
                    pt = ps_pj.tile([128, 2, 64], f32, tag="pj")
                    for kt in range(8):
                        nc.tensor.matmul(
                            out=pt,
                            lhsT=xT_sb[:, b, kt, st * 128:(st + 1) * 128],
                            rhs=wv_sb[:, kt, :],
                            start=(kt == 0), stop=(kt == 7))
                    nc.vector.tensor_copy(
                        out=Vs[:, b, st, :, 0:64], in_=pt)

            # ---- attention (softmax pipelined with PE) ----
            # During a stage's scores burst the PE is paced by exp on
            # ScalarE (~930ns/tile vs ~430ns/tile to produce): the two
            # score-psum buffers recycle at the exp rate.  To keep the PE
            # busy, exp-INDEPENDENT work (previous stage's AV, projections,
            # output projection) is drained from a pending queue between
            # score tiles.
            E_tiles = {}
            zsb_tiles = {}

            def scores_exp(b, j, pending):
                nkt = 4 * j + 4
                E = epool.tile([128, 16, 2, QCHUNK], bf16, tag="E")
                E_tiles[(b, j)] = E
                for t in range(nkt):
                    q0 = max(0, 128 * (t - 4 * j))
                    qs2 = slice(j * QCHUNK + q0, (j + 1) * QCHUNK)
                    sc = ps_sc.tile([128, 2, QCHUNK], f32, tag="sc")
                    for hh in range(2):
                        hs = slice(64 * hh, 64 * hh + 64)
                        nc.tensor.matmul(
                            out=sc[:, hh, q0:],
                            lhsT=Kr[hs, b, t * 128:(t + 1) * 128],
                            rhs=Qr[hs, b, qs2], start=True, stop=True)
                    nc.scalar.activation(
                        out=E[:, t, :, q0:], in_=sc[:, :, q0:],
                        func=AF.Exp, scale=1.0 / SCALE)
                    if t % 2 == 1 and pending:
                        pending.pop(0)()

            def mask(b, j):
                E = E_tiles[(b, j)]
                for dt in range(4):
                    t = 4 * j + dt
                    qb = slice(128 * dt, 128 * dt + 128)
                    nc.gpsimd.affine_select(
                        out=E[:, t, :, qb], in_=E[:, t, :, qb],
                        pattern=[[0, 2], [1, 128]], base=0,
                        channel_multiplier=-1,
                        compare_op=OP.is_ge, fill=0.0)

            def av_normalize_hh(b, j, hh):
                nkt = 4 * j + 4
                if hh == 0:
                    E = E_tiles[(b, j)]
                    zsb = zpool.tile([128, 4, 128], bf16, tag="zsb")
                    zsb_tiles[(b, j)] = zsb
                else:
                    E = E_tiles.pop((b, j))
                    zsb = zsb_tiles[(b, j)]
                hs = slice(64 * hh, 64 * hh + 64)
                z = ps_av.tile([65, 4, 128], f32, tag="av")
                for t in range(nkt):
                    q0 = max(0, 128 * (t - 4 * j))
                    nc.tensor.matmul(
                        out=z[:, q0 // 128:, :], lhsT=Vs[:, b, t, hh, :],
                        rhs=E[:, t, hh, q0:],
                        start=(t == 0), stop=(t == nkt - 1))
                d0 = den.tile([1, 4, 128], f32, tag="d0")
                nc.vector.tensor_copy(out=d0, in_=z[64:65, :, :])
                nc.vector.reciprocal_approx_fast(out=d0, in_=d0)
                rb = den.tile([64, 4, 128], f32, tag="rb")
                nc.gpsimd.partition_broadcast(out_ap=rb, in_ap=d0)
                nc.vector.tensor_tensor(
                    out=zsb[hs, :, :], in0=z[0:64, :, :], in1=rb,
                    op=OP.mult)

            # ---- z exchange + local output projection ----
            def a2a(j):
                for b in range(2):
                    zsb = zsb_tiles.pop((b, j))
                    for s4 in range(4):
                        nc.sync.dma_start(
                            out=z_send[j].ap()[4 * b + s4],
                            in_=zsb[:, s4, :])
                nc.gpsimd.collective_compute(
                    "AllToAll", mybir.AluOpType.bypass,
                    replica_groups=GROUPS_ALL,
                    ins=[z_send[j].ap().opt()],
                    outs=[z_recv[j].ap().opt()])

            def outproj(j):
                zall = zallp.tile([128, 8, 128], bf16, tag="zall")
                for s in range(8):
                    nc.gpsimd.dma_start(out=zall[:, s, :],
                                        in_=z_recv[j].ap()[s])
                for mc in range(2):
                    po = ps_pj.tile([128, 512], f32, tag="pj")
                    for kt in range(8):
                        nc.tensor.matmul(
                            out=po, lhsT=zall[:, kt, :],
                            rhs=wo_sb[:, kt, mc * 512:(mc + 1) * 512],
                            start=(kt == 0), stop=(kt == 7))
                    o_sb = osb.tile([128, 512], f16, tag="osb")
                    nc.vector.tensor_copy(out=o_sb, in_=po)
                    nc.sync.dma_start(
                        out=out_ext[j * 128:(j + 1) * 128,
                                    mc * 512:(mc + 1) * 512],
                        in_=o_sb)

            stages = [(b, j) for j in range(NCHUNK) for b in range(2)]
            for idx, (b, j) in enumerate(stages):
                pending = []
                if idx >= 1:
                    pb, pj = stages[idx - 1]
                    pending.append(
                        lambda pb=pb, pj=pj: av_normalize_hh(pb, pj, 0))
                    pending.append(
                        lambda pb=pb, pj=pj: av_normalize_hh(pb, pj, 1))
                    if pb == 1:
                        pending.append(lambda pj=pj: a2a(pj))
                if j < NCHUNK - 1:
                    pending.append(
                        lambda b=b, j=j: proj_chunk(b, j + 1, which=0))
                    pending.append(
                        lambda b=b, j=j: proj_chunk(b, j + 1, which=1))
                    pending.append(
                        lambda b=b, j=j: proj_v(b, j + 1, half=0))
                    pending.append(
                        lambda b=b, j=j: proj_v(b, j + 1, half=1))
                if j == 0:
                    proj_chunk(b, 0)
                    proj_v(b, 0)
                scores_exp(b, j, pending)
                for th in pending:
                    th()
                if idx >= 4 and stages[idx - 3][0] == 1:
                    outproj(stages[idx - 3][1])
                mask(b, j)
            av_normalize_hh(1, NCHUNK - 1, 0)
            av_normalize_hh(1, NCHUNK - 1, 1)
            a2a(NCHUNK - 1)
            outproj(NCHUNK - 2)
            outproj(NCHUNK - 1)

    nc.compile()
    return nc


def _get_built(with_qk_bias):
    key = bool(with_qk_bias)
    if key not in _BUILT:
        _BUILT[key] = _build(key)
    return _BUILT[key]


def _rope_tables():
    pos = np.arange(S, dtype=np.float64)
    dim = np.arange(DH // 2, dtype=np.float64)
    freq = ROT_BASE ** (dim / (DH / 2))
    freq = np.concatenate([freq, freq])                # [64]
    ang = pos[None, :] / freq[:, None]                 # [64, S]
    cos = np.cos(ang)
    sin = np.sin(ang)
    # sign of the rotate-half term folded into sin': rows 0..31 get -sin
    sinm = sin.copy()
    sinm[:DH // 2] *= -1.0
    cosT = np.tile(cos, (2, 1)).astype(BF)             # [128, S]
    sinT = np.tile(sinm, (2, 1)).astype(BF)
    return cosT, sinT


def kernel(x, W_Q, b_Q, W_K, b_K, W_V, b_V, W_O, b_O):
    from concourse.bass_utils import run_bass_kernel_spmd

    x = np.asarray(x)
    W_Q, W_K, W_V, W_O = (np.asarray(a) for a in (W_Q, W_K, W_V, W_O))
    b_Q, b_K, b_V, b_O = (np.asarray(a) for a in (b_Q, b_K, b_V, b_O))
    with_qk_bias = bool(np.any(b_Q) or np.any(b_K))
    nc = _get_built(with_qk_bias)

    cosT, sinT = _rope_tables()

    def wtile(w):            # [1024, C] -> [128, 8, C]
        c = w.shape[1]
        return np.ascontiguousarray(
            w.reshape(8, 128, c).transpose(1, 0, 2)).astype(BF)

    # x transposed per batch: [d, s]: d = kt*128 + p -> [p, kt, s]
    xT_host = np.stack([
        np.ascontiguousarray(
            x[b].T.reshape(8, 128, S).transpose(1, 0, 2)).astype(BF)
        for b in range(2)], axis=0)
    # W_O for ALL heads: slot s = heads (2s, 2s+1); identical on all cores
    wo_h = np.ascontiguousarray(
        np.concatenate([W_O[h] for h in range(NH)], axis=0)  # [1024, 1024]
        .reshape(8, 128, D).transpose(1, 0, 2)).astype(BF)

    in_maps = []
    for core in range(N_CORES):
        h0 = 2 * core
        wq_h = wtile(np.concatenate([W_Q[h0], W_Q[h0 + 1]], axis=1))
        wk_h = wtile(np.concatenate([W_K[h0], W_K[h0 + 1]], axis=1))
        wv_h = wtile(np.concatenate([W_V[h0], W_V[h0 + 1]], axis=1))
        m = {
            "xT": xT_host, "wq": wq_h, "wk": wk_h, "wv": wv_h, "wo": wo_h,
            "cosT": cosT, "sinTm": sinT,
        }
        if with_qk_bias:
            m["bq"] = np.concatenate(
                [b_Q[h0], b_Q[h0 + 1]]).astype(np.float32)[:, None]
            m["bk"] = np.concatenate(
                [b_K[h0], b_K[h0 + 1]]).astype(np.float32)[:, None]
        in_maps.append(m)

    global _last_in_maps
    _last_in_maps = in_maps
    res = run_bass_kernel_spmd(nc, in_maps, list(range(N_CORES)))

    out = np.empty((2, S, D), dtype=np.float32)
    for core in range(N_CORES):
        b, r = divmod(core, 4)
        shard = res.results[core]["out_shard"].astype(np.float32)
        for j in range(NCHUNK):
            out[b, QCHUNK * j + 128 * r: QCHUNK * j + 128 * (r + 1), :] = \
                shard[128 * j:128 * (j + 1)]

    # b_V shifts z by exactly b_V (softmax rows sum to 1); fold with b_O.
    corr = b_O.astype(np.float64).copy()
    if np.any(b_V):
        corr = corr + np.einsum("hd,hdm->m", b_V.astype(np.float64),
                                W_O.astype(np.float64))
    if np.any(corr):
        out = out + corr.astype(np.float32)
    return out


# revision 30
# speedup vs baseline: 1.2036x; 1.0106x over previous
"""Trainium2 Bass kernel for causal multi-head attention with NeoX RoPE.

Problem: x[2, 2048, 1024], 16 heads x d_head 64, rotary over all 64 dims,
causal softmax, output projection.

Sharding (v2): every core holds 2 heads ({2c, 2c+1}) and processes BOTH
batches.  The output projection is computed after a per-q-chunk 8-core
AllToAll of the normalized z shards: core c sends z[b, qsub, its 2 heads]
to the core that owns output rows (b, qsub); each core then contracts all
16 heads locally (W_O resident, slot s = heads {2s, 2s+1}) and writes its
own [128 x 1024] output rows per chunk.  This replaces the f16-partial
ReduceScatter chain of v1 (4x less collective traffic, no serial RS tail).

Per-core dataflow (batch b, 2 heads packed on 128 partitions):
  xT [b, d, s] (bf16, host-transposed)
  Q/K projections -> [128, s] via PE; RoPE as q*cos + flip(q)*sin' with the
    rotate-half flip fused into per-32-partition-block multiplies and the
    flip sign folded into the sin table.
  V projection -> Vs [s, h, 65] with a ones column (softmax denominator
    rides the AV matmul for free).
  Scores transposed: S_T[k, q] = kT.T @ qT per 128-k-tile; the two heads run
    as concurrent 64x128 PE row-tiles; exp on ScalarE (scale 1/8 folded in);
    causal mask via narrow GPSIMD affine_select on the 128x128 diagonal
    blocks only.
  AV: z[65, q] += V_aug.T @ E per k-tile; row 64 is the denominator.
  Normalize via reciprocal_approx_fast + partition_broadcast + multiply.
  AllToAll(z) per chunk, then out[q, m] = zall.T @ W_O locally.
"""

import numpy as np
import ml_dtypes

S = 2048
D = 1024
NH = 16
DH = 64
SCALE = 8.0
ROT_BASE = 10000.0
N_CORES = 8
QCHUNK = 512     # q chunk (free dim of score matmuls)
NCHUNK = S // QCHUNK
KTILE = 128
BF = ml_dtypes.bfloat16
GROUPS_ALL = [[0, 1, 2, 3, 4, 5, 6, 7]]

_BUILT = {}


def _build(with_qk_bias):
    import concourse.bass as bass
    import concourse.tile as tile
    from concourse import bacc, mybir

    f32 = mybir.dt.float32
    bf16 = mybir.dt.bfloat16
    f16 = mybir.dt.float16
    AF = mybir.ActivationFunctionType
    OP = mybir.AluOpType

    nc = bacc.Bacc("TRN2", target_bir_lowering=False, debug=False,
                   num_devices=N_CORES)

    xT = nc.dram_tensor("xT", [2, 128, 8, S], bf16, kind="ExternalInput").ap()
    wq = nc.dram_tensor("wq", [128, 8, 128], bf16, kind="ExternalInput").ap()
    wk = nc.dram_tensor("wk", [128, 8, 128], bf16, kind="ExternalInput").ap()
    wv = nc.dram_tensor("wv", [128, 8, 128], bf16, kind="ExternalInput").ap()
    wo = nc.dram_tensor("wo", [128, 8, D], bf16, kind="ExternalInput").ap()
    cosd = nc.dram_tensor("cosT", [128, S], bf16, kind="ExternalInput").ap()
    sind = nc.dram_tensor("sinTm", [128, S], bf16, kind="ExternalInput").ap()
    if with_qk_bias:
        bqd = nc.dram_tensor("bq", [128, 1], f32, kind="ExternalInput").ap()
        bkd = nc.dram_tensor("bk", [128, 1], f32, kind="ExternalInput").ap()

    z_send = [nc.dram_tensor(f"z_send{j}", [8, 128, 128], bf16)
              for j in range(NCHUNK)]
    z_recv = [nc.dram_tensor(f"z_recv{j}", [8, 128, 128], bf16)
              for j in range(NCHUNK)]
    out_ext = nc.dram_tensor("out_shard", [S // 4, D], f16,
                             kind="ExternalOutput").ap()

    with tile.TileContext(nc) as tc:
        with (
            tc.tile_pool(name="consts", bufs=1) as consts,
            tc.tile_pool(name="qk", bufs=1) as qkpool,
            tc.tile_pool(name="vsb", bufs=1) as vpool,
            tc.tile_pool(name="rope", bufs=2) as rope,
            tc.tile_pool(name="epool", bufs=2) as epool,
            tc.tile_pool(name="zpool", bufs=4) as zpool,
            tc.tile_pool(name="den", bufs=2) as den,
            tc.tile_pool(name="zail", bufs=2) as zallp,
            tc.tile_pool(name="osb", bufs=2) as osb,
            tc.tile_pool(name="ps_sc", bufs=2, space="PSUM") as ps_sc,
            tc.tile_pool(name="ps_av", bufs=2, space="PSUM") as ps_av,
            tc.tile_pool(name="ps_pj", bufs=2, space="PSUM") as ps_pj,
        ):
            wq_sb = consts.tile([128, 8, 128], bf16, tag="wq")
            nc.gpsimd.dma_start(out=wq_sb, in_=wq)
            wk_sb = consts.tile([128, 8, 128], bf16, tag="wk")
            nc.gpsimd.dma_start(out=wk_sb, in_=wk)
            cos_sb = consts.tile([128, S], bf16, tag="cos")
            nc.gpsimd.dma_start(out=cos_sb, in_=cosd)
            sin_sb = consts.tile([128, S], bf16, tag="sin")
            nc.gpsimd.dma_start(out=sin_sb, in_=sind)
            wv_sb = consts.tile([128, 8, 128], bf16, tag="wv")
            nc.gpsimd.dma_start(out=wv_sb, in_=wv)
            wo_sb = consts.tile([128, 8, D], bf16, tag="wo")
            nc.gpsimd.dma_start(out=wo_sb, in_=wo)

            xT_sb = consts.tile([128, 2, 8, S], bf16, tag="xT")
            # first-needed slices first: batch 0 cols 0:512, then the rest
            for kt in range(8):
                nc.sync.dma_start(out=xT_sb[:, 0, kt, 0:512],
                                  in_=xT[0][:, kt, 0:512])
            for kt in range(8):
                nc.sync.dma_start(out=xT_sb[:, 0, kt, 512:S],
                                  in_=xT[0][:, kt, 512:S])
            for kt in range(8):
                nc.sync.dma_start(out=xT_sb[:, 1, kt, :], in_=xT[1][:, kt, :])
            if with_qk_bias:
                bq_sb = consts.tile([128, 1], f32, tag="bq")
                nc.sync.dma_start(out=bq_sb, in_=bqd)
                bk_sb = consts.tile([128, 1], f32, tag="bk")
                nc.sync.dma_start(out=bk_sb, in_=bkd)

            warm = consts.tile([128, 8], f32, tag="warm")
            nc.vector.memset(warm, 0.0)
            nc.scalar.activation(out=warm, in_=warm, func=AF.Exp, scale=1.0)

            # Persistent rotated Q/K: [128 (=2-head pack), batch, s]
            Qr = qkpool.tile([128, 2, S], bf16, tag="Qr")
            Kr = qkpool.tile([128, 2, S], bf16, tag="Kr")
            # V with ones column: [s-part, batch, s-tile, head, 65]
            Vs = vpool.tile([128, 2, 16, 2, 65], bf16, tag="Vs")
            nc.vector.memset(Vs[:, :, :, :, 64:65], 1.0)

            # ---- per-chunk projections (interleaved with attention) ----
            def proj_chunk(b, c, which=None):
                cs = slice(c * QCHUNK, (c + 1) * QCHUNK)
                sel = ((wq_sb, "bq", Qr), (wk_sb, "bk", Kr))
                if which is not None:
                    sel = (sel[which],)
                for (wsb, bias_name, dst) in sel:
                    pt = ps_pj.tile([128, QCHUNK], f32, tag="pj")
                    for kt in range(8):
                        nc.tensor.matmul(
                            out=pt, lhsT=wsb[:, kt, :],
                            rhs=xT_sb[:, b, kt, cs],
                            start=(kt == 0), stop=(kt == 7))
                    if with_qk_bias:
                        bsb = bq_sb if bias_name == "bq" else bk_sb
                        nc.vector.tensor_scalar_add(
                            out=pt, in0=pt, scalar1=bsb[:, 0:1])
                    q_sb = rope.tile([128, QCHUNK], bf16, tag="ropeA")
                    nc.vector.tensor_copy(out=q_sb, in_=pt)
                    # q_rot = q*cos + flip(q)*sin' (sign folded into sin')
                    qf = rope.tile([128, QCHUNK], bf16, tag="ropeB")
                    for blk in range(4):
                        src = (blk ^ 1) * 32
                        nc.vector.tensor_copy(
                            out=qf[blk * 32:blk * 32 + 32, :],
                            in_=q_sb[src:src + 32, :])
                    qs = rope.tile([128, QCHUNK], bf16, tag="ropeC")
                    nc.vector.tensor_tensor(
                        out=qs, in0=qf, in1=sin_sb[:, cs], op=OP.mult)
                    qc = rope.tile([128, QCHUNK], bf16, tag="ropeB")
                    nc.vector.tensor_tensor(
                        out=qc, in0=q_sb, in1=cos_sb[:, cs], op=OP.mult)
                    nc.vector.tensor_tensor(
                        out=dst[:, b, cs], in0=qc, in1=qs, op=OP.add)

            def proj_v(b, c, half=None):
                sts = (range(4 * c, 4 * c + 4) if half is None else
                       range(4 * c + 2 * half, 4 * c + 2 * half + 2))
                for st in sts:
                    pt = ps_pj.tile([128, 2, 64], f32, tag="pj")
                    for kt in range(8):
                        nc.tensor.matmul(
                            out=pt,
                            lhsT=xT_sb[:, b, kt, st * 128:(st + 1) * 128],
                            rhs=wv_sb[:, kt, :],
                            start=(kt == 0), stop=(kt == 7))
                    nc.vector.tensor_copy(
                        out=Vs[:, b, st, :, 0:64], in_=pt)

            # ---- attention (softmax pipelined with PE) ----
            # During a stage's scores burst the PE is paced by exp on
            # ScalarE (~930ns/tile vs ~430ns/tile to produce): the two
            # score-psum buffers recycle at the exp rate.  To keep the PE
            # busy, exp-INDEPENDENT work (previous stage's AV, projections,
            # output projection) is drained from a pending queue between
            # score tiles.
            E_tiles = {}
            zsb_tiles = {}

            def scores_exp(b, j, pending):
                nkt = 4 * j + 4
                E = epool.tile([128, 16, 2, QCHUNK], bf16, tag="E")
                E_tiles[(b, j)] = E
                for t in range(nkt):
                    q0 = max(0, 128 * (t - 4 * j))
                    qs2 = slice(j * QCHUNK + q0, (j + 1) * QCHUNK)
                    sc = ps_sc.tile([128, 2, QCHUNK], f32, tag="sc")
                    for hh in range(2):
                        hs = slice(64 * hh, 64 * hh + 64)
                        nc.tensor.matmul(
                            out=sc[:, hh, q0:],
                            lhsT=Kr[hs, b, t * 128:(t + 1) * 128],
                            rhs=Qr[hs, b, qs2], start=True, stop=True)
                    nc.scalar.activation(
                        out=E[:, t, :, q0:], in_=sc[:, :, q0:],
                        func=AF.Exp, scale=1.0 / SCALE)
                    if t >= 4 * j:  # diagonal tile: causal mask (q >= k)
                        qb = slice(q0, q0 + 128)
                        nc.gpsimd.affine_select(
                            out=E[:, t, :, qb], in_=E[:, t, :, qb],
                            pattern=[[0, 2], [1, 128]], base=0,
                            channel_multiplier=-1,
                            compare_op=OP.is_ge, fill=0.0)
                    if t % 2 == 1 and pending:
                        pending.pop(0)()

            def av_normalize_hh(b, j, hh):
                nkt = 4 * j + 4
                if hh == 0:
                    E = E_tiles[(b, j)]
                    zsb = zpool.tile([128, 4, 128], bf16, tag="zsb")
                    zsb_tiles[(b, j)] = zsb
                else:
                    E = E_tiles.pop((b, j))
                    zsb = zsb_tiles[(b, j)]
                hs = slice(64 * hh, 64 * hh + 64)
                z = ps_av.tile([65, 4, 128], f32, tag="av")
                for t in range(nkt):
                    q0 = max(0, 128 * (t - 4 * j))
                    nc.tensor.matmul(
                        out=z[:, q0 // 128:, :], lhsT=Vs[:, b, t, hh, :],
                        rhs=E[:, t, hh, q0:],
                        start=(t == 0), stop=(t == nkt - 1))
                d0 = den.tile([1, 4, 128], f32, tag="d0")
                nc.vector.tensor_copy(out=d0, in_=z[64:65, :, :])
                nc.vector.reciprocal_approx_fast(out=d0, in_=d0)
                rb = den.tile([64, 4, 128], f32, tag="rb")
                nc.gpsimd.partition_broadcast(out_ap=rb, in_ap=d0)
                nc.vector.tensor_tensor(
                    out=zsb[hs, :, :], in0=z[0:64, :, :], in1=rb,
                    op=OP.mult)

            # ---- z exchange + local output projection ----
            def a2a(j):
                for b in range(2):
                    zsb = zsb_tiles.pop((b, j))
                    for s4 in range(4):
                        nc.sync.dma_start(
                            out=z_send[j].ap()[4 * b + s4],
                            in_=zsb[:, s4, :])
                nc.gpsimd.collective_compute(
                    "AllToAll", mybir.AluOpType.bypass,
                    replica_groups=GROUPS_ALL,
                    ins=[z_send[j].ap().opt()],
                    outs=[z_recv[j].ap().opt()])

            def outproj(j):
                zall = zallp.tile([128, 8, 128], bf16, tag="zall")
                for s in range(8):
                    nc.gpsimd.dma_start(out=zall[:, s, :],
                                        in_=z_recv[j].ap()[s])
                for mc in range(2):
                    po = ps_pj.tile([128, 512], f32, tag="pj")
                    for kt in range(8):
                        nc.tensor.matmul(
                            out=po, lhsT=zall[:, kt, :],
                            rhs=wo_sb[:, kt, mc * 512:(mc + 1) * 512],
                            start=(kt == 0), stop=(kt == 7))
                    o_sb = osb.tile([128, 512], f16, tag="osb")
                    nc.vector.tensor_copy(out=o_sb, in_=po)
                    nc.sync.dma_start(
                        out=out_ext[j * 128:(j + 1) * 128,
                                    mc * 512:(mc + 1) * 512],
                        in_=o_sb)

            stages = [(b, j) for j in range(NCHUNK) for b in range(2)]
            for idx, (b, j) in enumerate(stages):
                pending = []
                if idx >= 1:
                    pb, pj = stages[idx - 1]
                    pending.append(
                        lambda pb=pb, pj=pj: av_normalize_hh(pb, pj, 0))
                    pending.append(
                        lambda pb=pb, pj=pj: av_normalize_hh(pb, pj, 1))
                    if pb == 1:
                        pending.append(lambda pj=pj: a2a(pj))
                if j < NCHUNK - 1:
                    pending.append(
                        lambda b=b, j=j: proj_chunk(b, j + 1, which=0))
                    pending.append(
                        lambda b=b, j=j: proj_chunk(b, j + 1, which=1))
                    pending.append(
                        lambda b=b, j=j: proj_v(b, j + 1, half=0))
                    pending.append(
                        lambda b=b, j=j: proj_v(b, j + 1, half=1))
                if j == 0:
                    proj_chunk(b, 0)
                    proj_v(b, 0)
                if idx >= 5 and stages[idx - 4][0] == 1:
                    pending.append(
                        lambda jj=stages[idx - 4][1]: outproj(jj))
                scores_exp(b, j, pending)
                for th in pending:
                    th()
            av_normalize_hh(1, NCHUNK - 1, 0)
            av_normalize_hh(1, NCHUNK - 1, 1)
            a2a(NCHUNK - 1)
            outproj(NCHUNK - 2)
            outproj(NCHUNK - 1)

    nc.compile()
    return nc


def _get_built(with_qk_bias):
    key = bool(with_qk_bias)
    if key not in _BUILT:
        _BUILT[key] = _build(key)
    return _BUILT[key]


def _rope_tables():
    pos = np.arange(S, dtype=np.float64)
    dim = np.arange(DH // 2, dtype=np.float64)
    freq = ROT_BASE ** (dim / (DH / 2))
    freq = np.concatenate([freq, freq])                # [64]
    ang = pos[None, :] / freq[:, None]                 # [64, S]
    cos = np.cos(ang)
    sin = np.sin(ang)
    # sign of the rotate-half term folded into sin': rows 0..31 get -sin
    sinm = sin.copy()
    sinm[:DH // 2] *= -1.0
    cosT = np.tile(cos, (2, 1)).astype(BF)             # [128, S]
    sinT = np.tile(sinm, (2, 1)).astype(BF)
    return cosT, sinT


def kernel(x, W_Q, b_Q, W_K, b_K, W_V, b_V, W_O, b_O):
    from concourse.bass_utils import run_bass_kernel_spmd

    x = np.asarray(x)
    W_Q, W_K, W_V, W_O = (np.asarray(a) for a in (W_Q, W_K, W_V, W_O))
    b_Q, b_K, b_V, b_O = (np.asarray(a) for a in (b_Q, b_K, b_V, b_O))
    with_qk_bias = bool(np.any(b_Q) or np.any(b_K))
    nc = _get_built(with_qk_bias)

    cosT, sinT = _rope_tables()

    def wtile(w):            # [1024, C] -> [128, 8, C]
        c = w.shape[1]
        return np.ascontiguousarray(
            w.reshape(8, 128, c).transpose(1, 0, 2)).astype(BF)

    # x transposed per batch: [d, s]: d = kt*128 + p -> [p, kt, s]
    xT_host = np.stack([
        np.ascontiguousarray(
            x[b].T.reshape(8, 128, S).transpose(1, 0, 2)).astype(BF)
        for b in range(2)], axis=0)
    # W_O for ALL heads: slot s = heads (2s, 2s+1); identical on all cores
    wo_h = np.ascontiguousarray(
        np.concatenate([W_O[h] for h in range(NH)], axis=0)  # [1024, 1024]
        .reshape(8, 128, D).transpose(1, 0, 2)).astype(BF)

    in_maps = []
    for core in range(N_CORES):
        h0 = 2 * core
        wq_h = wtile(np.concatenate([W_Q[h0], W_Q[h0 + 1]], axis=1))
        wk_h = wtile(np.concatenate([W_K[h0], W_K[h0 + 1]], axis=1))
        wv_h = wtile(np.concatenate([W_V[h0], W_V[h0 + 1]], axis=1))
        m = {
            "xT": xT_host, "wq": wq_h, "wk": wk_h, "wv": wv_h, "wo": wo_h,
            "cosT": cosT, "sinTm": sinT,
        }
        if with_qk_bias:
            m["bq"] = np.concatenate(
                [b_Q[h0], b_Q[h0 + 1]]).astype(np.float32)[:, None]
            m["bk"] = np.concatenate(
                [b_K[h0], b_K[h0 + 1]]).astype(np.float32)[:, None]
        in_maps.append(m)

    global _last_in_maps
    _last_in_maps = in_maps
    res = run_bass_kernel_spmd(nc, in_maps, list(range(N_CORES)))

    out = np.empty((2, S, D), dtype=np.float32)
    for core in range(N_CORES):
        b, r = divmod(core, 4)
        shard = res.results[core]["out_shard"].astype(np.float32)
        for j in range(NCHUNK):
            out[b, QCHUNK * j + 128 * r: QCHUNK * j + 128 * (r + 1), :] = \
                shard[128 * j:128 * (j + 1)]

    # b_V shifts z by exactly b_V (softmax rows sum to 1); fold with b_O.
    corr = b_O.astype(np.float64).copy()
    if np.any(b_V):
        corr = corr + np.einsum("hd,hdm->m", b_V.astype(np.float64),
                                W_O.astype(np.float64))
    if np.any(corr):
        out = out + corr.astype(np.float32)
    return out


# revision 31
# speedup vs baseline: 1.2133x; 1.0081x over previous
"""Trainium2 Bass kernel for causal multi-head attention with NeoX RoPE.

Problem: x[2, 2048, 1024], 16 heads x d_head 64, rotary over all 64 dims,
causal softmax, output projection.

Sharding (v2): every core holds 2 heads ({2c, 2c+1}) and processes BOTH
batches.  The output projection is computed after a per-q-chunk 8-core
AllToAll of the normalized z shards: core c sends z[b, qsub, its 2 heads]
to the core that owns output rows (b, qsub); each core then contracts all
16 heads locally (W_O resident, slot s = heads {2s, 2s+1}) and writes its
own [128 x 1024] output rows per chunk.  This replaces the f16-partial
ReduceScatter chain of v1 (4x less collective traffic, no serial RS tail).

Per-core dataflow (batch b, 2 heads packed on 128 partitions):
  xT [b, d, s] (bf16, host-transposed)
  Q/K projections -> [128, s] via PE; RoPE as q*cos + flip(q)*sin' with the
    rotate-half flip fused into per-32-partition-block multiplies and the
    flip sign folded into the sin table.
  V projection -> Vs [s, h, 65] with a ones column (softmax denominator
    rides the AV matmul for free).
  Scores transposed: S_T[k, q] = kT.T @ qT per 128-k-tile; the two heads run
    as concurrent 64x128 PE row-tiles; exp on ScalarE (scale 1/8 folded in);
    causal mask via narrow GPSIMD affine_select on the 128x128 diagonal
    blocks only.
  AV: z[65, q] += V_aug.T @ E per k-tile; row 64 is the denominator.
  Normalize via reciprocal_approx_fast + partition_broadcast + multiply.
  AllToAll(z) per chunk, then out[q, m] = zall.T @ W_O locally.
"""

import numpy as np
import ml_dtypes

S = 2048
D = 1024
NH = 16
DH = 64
SCALE = 8.0
ROT_BASE = 10000.0
N_CORES = 8
QCHUNK = 512     # q chunk (free dim of score matmuls)
NCHUNK = S // QCHUNK
KTILE = 128
BF = ml_dtypes.bfloat16
GROUPS_ALL = [[0, 1, 2, 3, 4, 5, 6, 7]]

_BUILT = {}


def _build(with_qk_bias):
    import concourse.bass as bass
    import concourse.tile as tile
    from concourse import bacc, mybir

    f32 = mybir.dt.float32
    bf16 = mybir.dt.bfloat16
    f16 = mybir.dt.float16
    AF = mybir.ActivationFunctionType
    OP = mybir.AluOpType

    nc = bacc.Bacc("TRN2", target_bir_lowering=False, debug=False,
                   num_devices=N_CORES)

    xT = nc.dram_tensor("xT", [2, 128, 8, S], bf16, kind="ExternalInput").ap()
    wq = nc.dram_tensor("wq", [128, 8, 128], bf16, kind="ExternalInput").ap()
    wk = nc.dram_tensor("wk", [128, 8, 128], bf16, kind="ExternalInput").ap()
    wv = nc.dram_tensor("wv", [128, 8, 128], bf16, kind="ExternalInput").ap()
    wo = nc.dram_tensor("wo", [128, 8, D], bf16, kind="ExternalInput").ap()
    cosd = nc.dram_tensor("cosT", [128, S], bf16, kind="ExternalInput").ap()
    sind = nc.dram_tensor("sinTm", [128, S], bf16, kind="ExternalInput").ap()
    if with_qk_bias:
        bqd = nc.dram_tensor("bq", [128, 1], f32, kind="ExternalInput").ap()
        bkd = nc.dram_tensor("bk", [128, 1], f32, kind="ExternalInput").ap()

    z_send = [nc.dram_tensor(f"z_send{j}", [8, 128, 128], bf16)
              for j in range(NCHUNK)]
    z_recv = [nc.dram_tensor(f"z_recv{j}", [8, 128, 128], bf16)
              for j in range(NCHUNK)]
    out_ext = nc.dram_tensor("out_shard", [S // 4, D], f16,
                             kind="ExternalOutput").ap()

    with tile.TileContext(nc) as tc:
        with (
            tc.tile_pool(name="consts", bufs=1) as consts,
            tc.tile_pool(name="qk", bufs=1) as qkpool,
            tc.tile_pool(name="vsb", bufs=1) as vpool,
            tc.tile_pool(name="rope", bufs=2) as rope,
            tc.tile_pool(name="epool", bufs=2) as epool,
            tc.tile_pool(name="zpool", bufs=4) as zpool,
            tc.tile_pool(name="den", bufs=2) as den,
            tc.tile_pool(name="zail", bufs=2) as zallp,
            tc.tile_pool(name="osb", bufs=2) as osb,
            tc.tile_pool(name="ps_sc", bufs=2, space="PSUM") as ps_sc,
            tc.tile_pool(name="ps_av", bufs=2, space="PSUM") as ps_av,
            tc.tile_pool(name="ps_pj", bufs=2, space="PSUM") as ps_pj,
        ):
            wq_sb = consts.tile([128, 8, 128], bf16, tag="wq")
            nc.gpsimd.dma_start(out=wq_sb, in_=wq)
            wk_sb = consts.tile([128, 8, 128], bf16, tag="wk")
            nc.gpsimd.dma_start(out=wk_sb, in_=wk)
            cos_sb = consts.tile([128, S], bf16, tag="cos")
            nc.gpsimd.dma_start(out=cos_sb, in_=cosd)
            sin_sb = consts.tile([128, S], bf16, tag="sin")
            nc.gpsimd.dma_start(out=sin_sb, in_=sind)
            wv_sb = consts.tile([128, 8, 128], bf16, tag="wv")
            nc.gpsimd.dma_start(out=wv_sb, in_=wv)
            wo_sb = consts.tile([128, 8, D], bf16, tag="wo")
            nc.gpsimd.dma_start(out=wo_sb, in_=wo)

            xT_sb = consts.tile([128, 2, 8, S], bf16, tag="xT")
            # first-needed slices first: batch 0 cols 0:512, then the rest
            for kt in range(8):
                nc.sync.dma_start(out=xT_sb[:, 0, kt, 0:512],
                                  in_=xT[0][:, kt, 0:512])
            for kt in range(8):
                nc.sync.dma_start(out=xT_sb[:, 0, kt, 512:S],
                                  in_=xT[0][:, kt, 512:S])
            for kt in range(8):
                nc.sync.dma_start(out=xT_sb[:, 1, kt, :], in_=xT[1][:, kt, :])
            if with_qk_bias:
                bq_sb = consts.tile([128, 1], f32, tag="bq")
                nc.sync.dma_start(out=bq_sb, in_=bqd)
                bk_sb = consts.tile([128, 1], f32, tag="bk")
                nc.sync.dma_start(out=bk_sb, in_=bkd)

            warm = consts.tile([128, 8], f32, tag="warm")
            nc.vector.memset(warm, 0.0)
            nc.scalar.activation(out=warm, in_=warm, func=AF.Exp, scale=1.0)

            # Persistent rotated Q/K: [128 (=2-head pack), batch, s]
            Qr = qkpool.tile([128, 2, S], bf16, tag="Qr")
            Kr = qkpool.tile([128, 2, S], bf16, tag="Kr")
            # V with ones column: [s-part, batch, s-tile, head, 65]
            Vs = vpool.tile([128, 2, 16, 2, 65], bf16, tag="Vs")
            nc.vector.memset(Vs[:, :, :, :, 64:65], 1.0)

            # ---- per-chunk projections (interleaved with attention) ----
            def proj_chunk(b, c, which=None):
                cs = slice(c * QCHUNK, (c + 1) * QCHUNK)
                sel = ((wq_sb, "bq", Qr), (wk_sb, "bk", Kr))
                if which is not None:
                    sel = (sel[which],)
                for (wsb, bias_name, dst) in sel:
                    pt = ps_pj.tile([128, QCHUNK], f32, tag="pj")
                    for kt in range(8):
                        nc.tensor.matmul(
                            out=pt, lhsT=wsb[:, kt, :],
                            rhs=xT_sb[:, b, kt, cs],
                            start=(kt == 0), stop=(kt == 7))
                    if with_qk_bias:
                        bsb = bq_sb if bias_name == "bq" else bk_sb
                        nc.vector.tensor_scalar_add(
                            out=pt, in0=pt, scalar1=bsb[:, 0:1])
                    q_sb = rope.tile([128, QCHUNK], bf16, tag="ropeA")
                    nc.vector.tensor_copy(out=q_sb, in_=pt)
                    # q_rot = q*cos + flip(q)*sin' (sign folded into sin')
                    qf = rope.tile([128, QCHUNK], bf16, tag="ropeB")
                    for blk in range(4):
                        src = (blk ^ 1) * 32
                        nc.vector.tensor_copy(
                            out=qf[blk * 32:blk * 32 + 32, :],
                            in_=q_sb[src:src + 32, :])
                    qs = rope.tile([128, QCHUNK], bf16, tag="ropeC")
                    nc.vector.tensor_tensor(
                        out=qs, in0=qf, in1=sin_sb[:, cs], op=OP.mult)
                    qc = rope.tile([128, QCHUNK], bf16, tag="ropeB")
                    nc.vector.tensor_tensor(
                        out=qc, in0=q_sb, in1=cos_sb[:, cs], op=OP.mult)
                    nc.vector.tensor_tensor(
                        out=dst[:, b, cs], in0=qc, in1=qs, op=OP.add)

            def proj_v(b, c, half=None):
                sts = (range(4 * c, 4 * c + 4) if half is None else
                       range(4 * c + 2 * half, 4 * c + 2 * half + 2))
                for st in sts:
                    pt = ps_pj.tile([128, 2, 64], f32, tag="pj")
                    for kt in range(8):
                        nc.tensor.matmul(
                            out=pt,
                            lhsT=xT_sb[:, b, kt, st * 128:(st + 1) * 128],
                            rhs=wv_sb[:, kt, :],
                            start=(kt == 0), stop=(kt == 7))
                    nc.vector.tensor_copy(
                        out=Vs[:, b, st, :, 0:64], in_=pt)

            # ---- attention (softmax pipelined with PE) ----
            # During a stage's scores burst the PE is paced by exp on
            # ScalarE (~930ns/tile vs ~430ns/tile to produce): the two
            # score-psum buffers recycle at the exp rate.  To keep the PE
            # busy, exp-INDEPENDENT work (previous stage's AV, projections,
            # output projection) is drained from a pending queue between
            # score tiles.
            E_tiles = {}
            zsb_tiles = {}

            def scores_exp(b, j, pending):
                nkt = 4 * j + 4
                E = epool.tile([128, 16, 2, QCHUNK], bf16, tag="E")
                E_tiles[(b, j)] = E
                for t in range(nkt):
                    q0 = max(0, 128 * (t - 4 * j))
                    qs2 = slice(j * QCHUNK + q0, (j + 1) * QCHUNK)
                    sc = ps_sc.tile([128, 2, QCHUNK], f32, tag="sc")
                    for hh in range(2):
                        hs = slice(64 * hh, 64 * hh + 64)
                        nc.tensor.matmul(
                            out=sc[:, hh, q0:],
                            lhsT=Kr[hs, b, t * 128:(t + 1) * 128],
                            rhs=Qr[hs, b, qs2], start=True, stop=True)
                    nc.scalar.activation(
                        out=E[:, t, :, q0:], in_=sc[:, :, q0:],
                        func=AF.Exp, scale=1.0 / SCALE)
                    if t >= 4 * j:  # diagonal tile: causal mask (q >= k)
                        qb = slice(q0, q0 + 128)
                        nc.gpsimd.affine_select(
                            out=E[:, t, :, qb], in_=E[:, t, :, qb],
                            pattern=[[0, 2], [1, 128]], base=0,
                            channel_multiplier=-1,
                            compare_op=OP.is_ge, fill=0.0)
                    if t % 2 == 1 and pending:
                        pending.pop(0)()

            def av_normalize_hh(b, j, hh):
                nkt = 4 * j + 4
                if hh == 0:
                    E = E_tiles[(b, j)]
                    zsb = zpool.tile([128, 4, 128], bf16, tag="zsb")
                    zsb_tiles[(b, j)] = zsb
                else:
                    E = E_tiles.pop((b, j))
                    zsb = zsb_tiles[(b, j)]
                hs = slice(64 * hh, 64 * hh + 64)
                z = ps_av.tile([65, 4, 128], f32, tag="av")
                for t in range(nkt):
                    q0 = max(0, 128 * (t - 4 * j))
                    nc.tensor.matmul(
                        out=z[:, q0 // 128:, :], lhsT=Vs[:, b, t, hh, :],
                        rhs=E[:, t, hh, q0:],
                        start=(t == 0), stop=(t == nkt - 1))
                d0 = den.tile([1, 4, 128], f32, tag="d0")
                nc.vector.tensor_copy(out=d0, in_=z[64:65, :, :])
                nc.vector.reciprocal_approx_fast(out=d0, in_=d0)
                rb = den.tile([64, 4, 128], f32, tag="rb")
                nc.gpsimd.partition_broadcast(out_ap=rb, in_ap=d0)
                nc.vector.tensor_tensor(
                    out=zsb[hs, :, :], in0=z[0:64, :, :], in1=rb,
                    op=OP.mult)

            # ---- z exchange + local output projection ----
            def a2a(j):
                for b in range(2):
                    zsb = zsb_tiles.pop((b, j))
                    for s4 in range(4):
                        nc.sync.dma_start(
                            out=z_send[j].ap()[4 * b + s4],
                            in_=zsb[:, s4, :])
                nc.gpsimd.collective_compute(
                    "AllToAll", mybir.AluOpType.bypass,
                    replica_groups=GROUPS_ALL,
                    ins=[z_send[j].ap().opt()],
                    outs=[z_recv[j].ap().opt()])

            def outproj(j):
                zall = zallp.tile([128, 8, 128], bf16, tag="zall")
                for s in range(8):
                    nc.gpsimd.dma_start(out=zall[:, s, :],
                                        in_=z_recv[j].ap()[s])
                po0 = ps_pj.tile([128, 512], f32, tag="pj")
                po1 = ps_pj.tile([128, 512], f32, tag="pj")
                # kt-outer so each zall k-slice is loaded into the PE once
                # and reused by both output-column halves
                for kt in range(8):
                    for mc, po in ((0, po0), (1, po1)):
                        nc.tensor.matmul(
                            out=po, lhsT=zall[:, kt, :],
                            rhs=wo_sb[:, kt, mc * 512:(mc + 1) * 512],
                            start=(kt == 0), stop=(kt == 7),
                            skip_group_check=True)
                for mc, po in ((0, po0), (1, po1)):
                    o_sb = osb.tile([128, 512], f16, tag="osb")
                    nc.vector.tensor_copy(out=o_sb, in_=po)
                    nc.sync.dma_start(
                        out=out_ext[j * 128:(j + 1) * 128,
                                    mc * 512:(mc + 1) * 512],
                        in_=o_sb)

            stages = [(b, j) for j in range(NCHUNK) for b in range(2)]
            for idx, (b, j) in enumerate(stages):
                pending = []
                if idx >= 1:
                    pb, pj = stages[idx - 1]
                    pending.append(
                        lambda pb=pb, pj=pj: av_normalize_hh(pb, pj, 0))
                    pending.append(
                        lambda pb=pb, pj=pj: av_normalize_hh(pb, pj, 1))
                    if pb == 1:
                        pending.append(lambda pj=pj: a2a(pj))
                if j < NCHUNK - 1:
                    pending.append(
                        lambda b=b, j=j: proj_chunk(b, j + 1, which=0))
                    pending.append(
                        lambda b=b, j=j: proj_chunk(b, j + 1, which=1))
                    pending.append(
                        lambda b=b, j=j: proj_v(b, j + 1, half=0))
                    pending.append(
                        lambda b=b, j=j: proj_v(b, j + 1, half=1))
                if j == 0:
                    proj_chunk(b, 0)
                    proj_v(b, 0)
                if idx >= 5 and stages[idx - 4][0] == 1:
                    pending.append(
                        lambda jj=stages[idx - 4][1]: outproj(jj))
                scores_exp(b, j, pending)
                for th in pending:
                    th()
            av_normalize_hh(1, NCHUNK - 1, 0)
            av_normalize_hh(1, NCHUNK - 1, 1)
            a2a(NCHUNK - 1)
            outproj(NCHUNK - 2)
            outproj(NCHUNK - 1)

    nc.compile()
    return nc


def _get_built(with_qk_bias):
    key = bool(with_qk_bias)
    if key not in _BUILT:
        _BUILT[key] = _build(key)
    return _BUILT[key]


def _rope_tables():
    pos = np.arange(S, dtype=np.float64)
    dim = np.arange(DH // 2, dtype=np.float64)
    freq = ROT_BASE ** (dim / (DH / 2))
    freq = np.concatenate([freq, freq])                # [64]
    ang = pos[None, :] / freq[:, None]                 # [64, S]
    cos = np.cos(ang)
    sin = np.sin(ang)
    # sign of the rotate-half term folded into sin': rows 0..31 get -sin
    sinm = sin.copy()
    sinm[:DH // 2] *= -1.0
    cosT = np.tile(cos, (2, 1)).astype(BF)             # [128, S]
    sinT = np.tile(sinm, (2, 1)).astype(BF)
    return cosT, sinT


def kernel(x, W_Q, b_Q, W_K, b_K, W_V, b_V, W_O, b_O):
    from concourse.bass_utils import run_bass_kernel_spmd

    x = np.asarray(x)
    W_Q, W_K, W_V, W_O = (np.asarray(a) for a in (W_Q, W_K, W_V, W_O))
    b_Q, b_K, b_V, b_O = (np.asarray(a) for a in (b_Q, b_K, b_V, b_O))
    with_qk_bias = bool(np.any(b_Q) or np.any(b_K))
    nc = _get_built(with_qk_bias)

    cosT, sinT = _rope_tables()

    def wtile(w):            # [1024, C] -> [128, 8, C]
        c = w.shape[1]
        return np.ascontiguousarray(
            w.reshape(8, 128, c).transpose(1, 0, 2)).astype(BF)

    # x transposed per batch: [d, s]: d = kt*128 + p -> [p, kt, s]
    xT_host = np.stack([
        np.ascontiguousarray(
            x[b].T.reshape(8, 128, S).transpose(1, 0, 2)).astype(BF)
        for b in range(2)], axis=0)
    # W_O for ALL heads: slot s = heads (2s, 2s+1); identical on all cores
    wo_h = np.ascontiguousarray(
        np.concatenate([W_O[h] for h in range(NH)], axis=0)  # [1024, 1024]
        .reshape(8, 128, D).transpose(1, 0, 2)).astype(BF)

    in_maps = []
    for core in range(N_CORES):
        h0 = 2 * core
        wq_h = wtile(np.concatenate([W_Q[h0], W_Q[h0 + 1]], axis=1))
        wk_h = wtile(np.concatenate([W_K[h0], W_K[h0 + 1]], axis=1))
        wv_h = wtile(np.concatenate([W_V[h0], W_V[h0 + 1]], axis=1))
        m = {
            "xT": xT_host, "wq": wq_h, "wk": wk_h, "wv": wv_h, "wo": wo_h,
            "cosT": cosT, "sinTm": sinT,
        }
        if with_qk_bias:
            m["bq"] = np.concatenate(
                [b_Q[h0], b_Q[h0 + 1]]).astype(np.float32)[:, None]
            m["bk"] = np.concatenate(
                [b_K[h0], b_K[h0 + 1]]).astype(np.float32)[:, None]
        in_maps.append(m)

    global _last_in_maps
    _last_in_maps = in_maps
    res = run_bass_kernel_spmd(nc, in_maps, list(range(N_CORES)))

    out = np.empty((2, S, D), dtype=np.float32)
    for core in range(N_CORES):
        b, r = divmod(core, 4)
        shard = res.results[core]["out_shard"].astype(np.float32)
        for j in range(NCHUNK):
            out[b, QCHUNK * j + 128 * r: QCHUNK * j + 128 * (r + 1), :] = \
                shard[128 * j:128 * (j + 1)]

    # b_V shifts z by exactly b_V (softmax rows sum to 1); fold with b_O.
    corr = b_O.astype(np.float64).copy()
    if np.any(b_V):
        corr = corr + np.einsum("hd,hdm->m", b_V.astype(np.float64),
                                W_O.astype(np.float64))
    if np.any(corr):
        out = out + corr.astype(np.float32)
    return out
